# revision 7
# baseline (speedup 1.0000x reference)
"""Swin-style windowed-attention block (LN->W-MSA->residual->LN->MLP->residual)
for TRN2, data-parallel over batch across 8 NeuronCores.

Design (vs. the f32/bf16 channel-major baseline):
- 3-stage skewed software pipeline (A=load/LN1/qkv, B=attention/proj/LN2,
  C=MLP/store) emitted A0 A1 B0 | A2 B1 C0 | A3 B2 C1 | B3 C2 | C3 so the
  ACT engine's exp and gelu streams stay back-to-back across images.
- fp8e4m3 DoubleRow matmuls (2 k-tiles per instruction, 0.5 cyc/row) for
  qkv/v/proj/mlp1/mlp2; weights scaled x64 host-side into DoubleRow pair
  tiles; descales folded into activation scale slots (exp scale 1/4096
  absorbs the q64*k64 scale, gelu scale 1/64, affine_then_add scale 1/64).
- bf16 residual stream (4x DVE rate on the LN square/z passes); fp8 q/k
  tiles (halves SBUF + DMA traffic for the qh/kh head rearrange).
- relative-position bias + window mask folded into the score PSUM via one
  identity-stationary matmul per tile (off-window = -20 -> exp ~ 0), so
  softmax is exp-only on ACT with no mask multiply on DVE.
- k-bias dropped (cancels in softmax over keys); v-bias folded into the
  proj bias (softmax weights sum to 1); LN gamma/beta folded into the
  following matmul weights/biases.
- LayerNorm: ones-matmul stats on PE, Square/Sqrt+reciprocal soup (one
  act-table set; Gelu/Exp/Sqrt sets cost 1.28us per switch), r/mur rows
  broadcast across partitions via gpsimd partition_broadcast (input must
  be on partition 0 - the Q7 kernel ignores the AP base partition).
- residual updates via the fused affine_then_add custom DVE op; window
  permutation via 4-dim strided access patterns on DVE copies; casting
  DMAs (f32->bf16 loads, bf16->fp8 z/gelu quantization, bf16->f32 store)
  batched on the gpsimd SWDGE queue.
Note: TensorScalarPtr/TensorTensor ops do NOT lower for the Pool engine
(walrus ISA check) even though CoreSim accepts them - Pool runs only DMAs,
partition_broadcast and real ISA library ops.
"""
import sys
import numpy as np
import ml_dtypes

sys.path.insert(0, "/opt/trn_rl_repo")

import concourse.bass as bass
import concourse.bacc as bacc
import concourse.tile as tile
from concourse import mybir
from concourse.bass_utils import run_bass_kernel_spmd

F32 = mybir.dt.float32
BF16 = mybir.dt.bfloat16
FP8 = mybir.dt.float8e4
AF = mybir.ActivationFunctionType
ALU = mybir.AluOpType
DR = mybir.MatmulPerfMode.DoubleRow
BF = ml_dtypes.bfloat16
E4 = ml_dtypes.float8_e4m3

B, C, H, W = 32, 512, 32, 32
NH, WS = 16, 4
HD = C // NH
N = WS * WS
EPS = 1e-5
MLP_H = 4 * C
NCORES = 8
BI = B // NCORES
T = H * W

WSC = 64.0
ISC = 1.0 / WSC

_cache = {}


def _relative_position_index(ws):
    coords = np.stack(np.meshgrid(np.arange(ws), np.arange(ws), indexing="ij"))
    cf = coords.reshape(2, -1)
    rel = cf[:, :, None] - cf[:, None, :]
    rel = rel.transpose(1, 2, 0).astype(np.int64)
    rel[:, :, 0] += ws - 1
    rel[:, :, 1] += ws - 1
    rel[:, :, 0] *= 2 * ws - 1
    return rel.sum(-1)


def _ap(t, off, dims):
    return bass.AP(tensor=t.tensor, offset=t.offset + off,
                   ap=[t.ap[0]] + [list(d) for d in dims])


def _bc(t, n):
    return bass.AP(tensor=t.tensor, offset=t.offset,
                   ap=[list(t.ap[0]), [0, n]])


def _build_program():
    nc = bacc.Bacc("TRN2", target_bir_lowering=False, debug=False,
                   enable_asserts=True, num_devices=NCORES)

    def din(name, shape, dt):
        return nc.dram_tensor(name, shape, dt, kind="ExternalInput").ap()

    x_d = din("x", (BI, C, H, W), F32)
    wqkv_d = din("wqkv", (2, 128, 2 * 3 * C), FP8)
    wproj_d = din("wproj", (2, 128, 2 * C), FP8)
    wm1_d = din("wm1", (2, 128, 2 * MLP_H), FP8)
    wm2_d = din("wm2", (8, 128, 2 * C), FP8)
    dq64_d = din("dq64", (128, 4), F32)
    pb_d = din("pb", (128, 4), F32)
    d1_d = din("d1", (128, 16), F32)
    b2_d = din("b2", (128, 4), F32)
    mask_d = din("mask", (128, 4 * 512), BF16)
    idm_d = din("idm", (128, 128), BF16)

    out_d = nc.dram_tensor("out", (BI, C, H, W), F32,
                           kind="ExternalOutput").ap()

    with tile.TileContext(nc) as tc:
        with tc.tile_pool(name="sb", bufs=1) as sb, \
             tc.tile_pool(name="ps", bufs=1, space="PSUM") as ps:

            # ---------------- resident constants ----------------
            wqkv = []
            for p in range(2):
                t = sb.tile([128, 2 * 3 * C], FP8, tag="wqkv", bufs=2,
                            name=f"wqkv{p}")
                nc.sync.dma_start(out=t, in_=wqkv_d[p])
                wqkv.append(t)
            wproj = []
            for p in range(2):
                t = sb.tile([128, 2 * C], FP8, tag="wproj", bufs=2,
                            name=f"wproj{p}")
                nc.sync.dma_start(out=t, in_=wproj_d[p])
                wproj.append(t)
            wm1 = []
            for p in range(2):
                t = sb.tile([128, 2 * MLP_H], FP8, tag="wm1", bufs=2,
                            name=f"wm1{p}")
                nc.sync.dma_start(out=t, in_=wm1_d[p])
                wm1.append(t)
            wm2 = []
            for p in range(8):
                t = sb.tile([128, 2 * C], FP8, tag="wm2", bufs=8,
                            name=f"wm2{p}")
                nc.sync.dma_start(out=t, in_=wm2_d[p])
                wm2.append(t)
            mask_t = sb.tile([128, 4 * 512], BF16, tag="mask", name="mask_t")
            nc.sync.dma_start(out=mask_t, in_=mask_d)
            idm = sb.tile([128, 128], BF16, tag="idm", name="idm")
            nc.sync.dma_start(out=idm, in_=idm_d)
            dq_t = sb.tile([128, 4], F32, tag="dq", name="dq_t")
            nc.sync.dma_start(out=dq_t, in_=dq64_d)
            pb_t = sb.tile([128, 4], F32, tag="pbt", name="pb_t")
            nc.sync.dma_start(out=pb_t, in_=pb_d)
            d1_t = sb.tile([128, 16], F32, tag="d1t", name="d1_t")
            nc.sync.dma_start(out=d1_t, in_=d1_d)
            b2_t = sb.tile([128, 4], F32, tag="b2t", name="b2_t")
            nc.sync.dma_start(out=b2_t, in_=b2_d)
            onesb = sb.tile([128, 1], BF16, tag="onesb", name="onesb")
            nc.vector.memset(onesb, 1.0)
            onesf = sb.tile([128, 1], F32, tag="onesf", name="onesf")
            nc.vector.memset(onesf, 1.0)
            epsb = sb.tile([1, 1], F32, tag="epsb", name="epsb")
            nc.vector.memset(epsb, EPS)

            WIN8 = [[128, 8], [16, 8], [4, 4], [1, 4]]
            RAS8 = [[128, 8], [4, 8], [32, 4], [1, 4]]

            def layernorm(xc, sfx):
                rowsb = sb.tile([1, 1024], BF16, tag="rowsb", bufs=2,
                                name=f"rowsb_{sfx}")
                rowsm = sb.tile([1, 1024], BF16, tag="rowsm", bufs=2,
                                name=f"rowsm_{sfx}")
                x2 = {}
                for hh in range(2):
                    for c4 in range(4):
                        x2t = sb.tile([128, 512], BF16, tag="x2", bufs=8,
                                      name=f"x2_{sfx}_{hh}_{c4}")
                        nc.vector.tensor_mul(
                            x2t, xc[c4][:, 512 * hh:512 * (hh + 1)],
                            xc[c4][:, 512 * hh:512 * (hh + 1)])
                        x2[(hh, c4)] = x2t
                for hh in range(2):
                    sx = ps.tile([1, 512], F32, tag="av", bufs=2,
                                 name=f"sx_{sfx}_{hh}")
                    sx2 = ps.tile([1, 512], F32, tag="av", bufs=2,
                                  name=f"sx2_{sfx}_{hh}")
                    sl0 = slice(512 * hh, 512 * (hh + 1))
                    for c4 in range(4):
                        nc.tensor.matmul(sx[0:1, :], onesb,
                                         xc[c4][:, sl0],
                                         start=(c4 == 0), stop=(c4 == 3))
                    for c4 in range(4):
                        nc.tensor.matmul(sx2[0:1, :], onesb, x2[(hh, c4)],
                                         start=(c4 == 0), stop=(c4 == 3))
                    sl = slice(512 * hh, 512 * (hh + 1))
                    with tc.high_priority():
                        mu2 = sb.tile([1, 512], F32, tag="tsq", bufs=2,
                                      name=f"mu2_{sfx}_{hh}")
                        nc.scalar.activation(mu2, sx[0:1, :], AF.Square,
                                             scale=1.0 / C)
                        varc = sb.tile([1, 512], F32, tag="varc", bufs=2,
                                       name=f"varc_{sfx}_{hh}")
                        nc.vector.scalar_tensor_tensor(
                            varc, sx2[0:1, :], 1.0 / C, mu2,
                            ALU.mult, ALU.subtract)
                        sdev = sb.tile([1, 512], F32, tag="rln", bufs=2,
                                       name=f"sdev_{sfx}_{hh}")
                        nc.scalar.activation(sdev, varc, AF.Sqrt, bias=epsb)
                        with nc.allow_low_precision(
                                reason="bf16 r/mur rows are intentional"):
                            nc.vector.reciprocal(rowsb[0:1, sl], sdev)
                            nc.vector.scalar_tensor_tensor(
                                rowsm[0:1, sl], sx[0:1, :], 1.0 / C,
                                rowsb[0:1, sl], ALU.mult, ALU.mult)
                rm_bc = sb.tile([128, 2048], BF16, tag="rmbc", bufs=2,
                                name=f"rmbc_{sfx}")
                nc.gpsimd.partition_broadcast(rm_bc[:, 0:1024],
                                              rowsb[0:1, :], channels=128)
                nc.gpsimd.partition_broadcast(rm_bc[:, 1024:2048],
                                              rowsm[0:1, :], channels=128)
                return rm_bc[:, 0:1024], rm_bc[:, 1024:2048]

            def z_quant(xc, r_bc, mur_bc, sfx):
                z8 = sb.tile([128, 4096], FP8, tag="z8p", bufs=3,
                             name=f"z8_{sfx}")
                for hf in range(2):
                    zb = sb.tile([128, 2048], BF16, tag="zb", bufs=2,
                                 name=f"zb_{sfx}_{hf}")
                    for cc in range(2):
                        c4 = 2 * hf + cc
                        t1 = sb.tile([128, 1024], BF16, tag="zt1", bufs=1,
                                     name=f"t1_{sfx}_{c4}")
                        nc.vector.tensor_mul(t1, xc[c4], r_bc)
                        nc.vector.tensor_tensor(
                            out=zb[:, 1024 * cc:1024 * (cc + 1)],
                            in0=t1, in1=mur_bc, op=ALU.subtract)
                    nc.gpsimd.dma_start(
                        out=z8[:, 2048 * hf:2048 * (hf + 1)], in_=zb)
                return z8

            # ---------------- stage A: load + LN1 + qkv ----------------
            def stage_a(img):
                xc = []
                for hf in range(2):
                    xrw = sb.tile([128, 2048], BF16, tag="xraw", bufs=3,
                                  name=f"xr_{img}_{hf}")
                    nc.gpsimd.dma_start(
                        out=xrw,
                        in_=bass.AP(
                            tensor=x_d.tensor,
                            offset=x_d.offset + img * C * H * W
                            + hf * 2 * 131072,
                            ap=[[1024, 128], [131072, 2], [1, 1024]]))
                    for cc in range(2):
                        c4 = 2 * hf + cc
                        xt = sb.tile([128, 1024], BF16, tag="xc", bufs=12,
                                     name=f"x_{img}_{c4}")
                        nc.vector.tensor_copy(_ap(xt, 0, WIN8),
                                              _ap(xrw, 1024 * cc, RAS8))
                        xc.append(xt)
                with tc.high_priority():
                    r_bc, mur_bc = layernorm(xc, f"l1_{img}")
                z8 = z_quant(xc, r_bc, mur_bc, f"l1_{img}")

                # qkv: q/k fp8 f-tiles (channel-major)
                qk = {}
                for fi in (0, 4, 1, 5, 2, 6, 3, 7):
                    qkt = sb.tile([128, 1024], FP8, tag="qk", bufs=13,
                                  name=f"qk_{img}_{fi}")
                    for th in range(2):
                        mm = ps.tile([128, 512], F32, tag="mm", bufs=2,
                                     name=f"qkp_{img}_{fi}_{th}")
                        for p in range(2):
                            nc.tensor.matmul(
                                mm,
                                _ap(wqkv[p], 128 * fi, [[1536, 2], [1, 128]]),
                                _ap(z8, 2048 * p + 512 * th,
                                    [[1024, 2], [1, 512]]),
                                start=(p == 0), stop=(p == 1), perf_mode=DR)
                        with tc.high_priority():
                            if fi < 4:
                                nc.scalar.activation(
                                    qkt[:, 512 * th:512 * (th + 1)], mm,
                                    AF.Identity, bias=dq_t[:, fi:fi + 1])
                            else:
                                nc.vector.tensor_copy(
                                    qkt[:, 512 * th:512 * (th + 1)], mm)
                    qk[fi] = qkt

                # v (token-major bf16 + interleaved ones column)
                vaug = []
                for g in range(8):
                    mm = ps.tile([128, 512], F32, tag="mm", bufs=2,
                                 name=f"vp_{img}_{g}")
                    for p in range(2):
                        nc.tensor.matmul(
                            mm,
                            _ap(z8, 2048 * p + 128 * g, [[1024, 2], [1, 128]]),
                            _ap(wqkv[p], 2 * C, [[1536, 2], [1, 512]]),
                            start=(p == 0), stop=(p == 1), perf_mode=DR)
                    va = sb.tile([128, 33 * NH], BF16, tag="vaug", bufs=16,
                                 name=f"va_{img}_{g}")
                    nc.vector.memset(_ap(va, 32, [[33, NH]]), 1.0)
                    nc.vector.tensor_scalar_mul(
                        _ap(va, 0, [[33, NH], [1, 32]]),
                        _ap(mm, 0, [[32, NH], [1, 32]]), ISC)
                    vaug.append(va)
                return xc, qk, vaug

            # ------- stage B: attention + proj + LN2 + z28 -------
            def stage_b(img, xc, qk, vaug):
                atc = [sb.tile([128, 512], BF16, tag="atc", bufs=8,
                               name=f"atc_{img}_{g}") for g in range(8)]
                for qt in range(4):
                    qh = sb.tile([32, 4096], FP8, tag="qh", bufs=3,
                                 name=f"qh_{img}_{qt}")
                    kh = sb.tile([32, 4096], FP8, tag="kh", bufs=3,
                                 name=f"kh_{img}_{qt}")
                    for b4 in range(4):
                        nc.sync.dma_start(
                            out=qh[0:32, 1024 * b4:1024 * (b4 + 1)],
                            in_=qk[qt][32 * b4:32 * (b4 + 1), :])
                        nc.sync.dma_start(
                            out=kh[0:32, 1024 * b4:1024 * (b4 + 1)],
                            in_=qk[4 + qt][32 * b4:32 * (b4 + 1), :])
                    for g in range(8):
                        stp = ps.tile([128, 512], F32, tag="st", bufs=4,
                                      name=f"stp_{img}_{qt}_{g}")
                        nc.tensor.matmul(
                            stp, idm, mask_t[:, 512 * qt:512 * (qt + 1)],
                            start=True, stop=False, skip_group_check=True)
                        for b4 in range(4):
                            sl = slice(1024 * b4 + 128 * g,
                                       1024 * b4 + 128 * (g + 1))
                            nc.tensor.matmul(
                                stp[:, 128 * b4:128 * (b4 + 1)],
                                kh[0:32, sl], qh[0:32, sl],
                                start=False, stop=(b4 == 3),
                                skip_group_check=True)
                        pt = sb.tile([128, 512], BF16, tag="pt", bufs=3,
                                     name=f"pt_{img}_{qt}_{g}")
                        nc.scalar.activation(pt, stp, AF.Exp,
                                             scale=1.0 / (WSC * WSC))
                        av = ps.tile([128, 132], F32, tag="av", bufs=2,
                                     name=f"av_{img}_{qt}_{g}")
                        for b4 in range(4):
                            h = 4 * qt + b4
                            nc.tensor.matmul(
                                av[:, 33 * b4:33 * (b4 + 1)],
                                pt[:, 128 * b4:128 * (b4 + 1)],
                                vaug[g][:, 33 * h:33 * (h + 1)],
                                start=True, stop=True)
                        rec = sb.tile([128, 4], F32, tag="rec", bufs=4,
                                      name=f"rec_{img}_{qt}_{g}")
                        nc.vector.reciprocal(rec, _ap(av, 32, [[33, 4]]))
                        nc.vector.tensor_tensor(
                            out=_ap(atc[g], 128 * qt, [[32, 4], [1, 32]]),
                            in0=_ap(av, 0, [[33, 4], [1, 32]]),
                            in1=_ap(rec, 0, [[1, 4], [0, 32]]),
                            op=ALU.mult)

                actn8 = [sb.tile([128, 2048], FP8, tag="actn", bufs=2,
                                 name=f"actn_{img}_{p}") for p in range(2)]
                for fp in range(4):
                    for Q in range(2):
                        tp = ps.tile([128, 512], BF16, tag="mm", bufs=2,
                                     name=f"tp_{img}_{fp}_{Q}")
                        for gq in range(4):
                            g = 4 * Q + gq
                            nc.tensor.transpose(
                                tp[:, 128 * gq:128 * (gq + 1)],
                                atc[g][:, 128 * fp:128 * (fp + 1)], idm)
                        dst = actn8[fp // 2][:, 1024 * (fp % 2) + 512 * Q:
                                             1024 * (fp % 2) + 512 * (Q + 1)]
                        if fp % 2 == 0:
                            nc.vector.tensor_copy(dst, tp)
                        else:
                            nc.scalar.copy(dst, tp)

                for th in range(2):
                    for fo in range(4):
                        mm = ps.tile([128, 512], F32, tag="mm", bufs=2,
                                     name=f"pj_{img}_{fo}_{th}")
                        for p in range(2):
                            nc.tensor.matmul(
                                mm,
                                _ap(wproj[p], 128 * fo, [[512, 2], [1, 128]]),
                                _ap(actn8[p], 512 * th, [[1024, 2], [1, 512]]),
                                start=(p == 0), stop=(p == 1), perf_mode=DR)
                        xv = xc[fo][:, 512 * th:512 * (th + 1)]
                        nc.vector.affine_then_add(
                            xv, mm, xv, scale=ISC,
                            bias=pb_t[:, fo:fo + 1])

                with tc.high_priority():
                    r2_bc, mur2_bc = layernorm(xc, f"l2_{img}")
                z28 = z_quant(xc, r2_bc, mur2_bc, f"l2_{img}")
                return z28

            # ---------------- stage C: MLP + store ----------------
            def stage_c(img, xc, z28):
                xout = [sb.tile([128, 1024], BF16, tag="xout", bufs=4,
                                name=f"xo_{img}_{c4}") for c4 in range(4)]
                for th in range(2):
                    g8 = []
                    for j in range(8):
                        gt = sb.tile([128, 1024], FP8, tag="g8", bufs=8,
                                     name=f"g8_{img}_{th}_{j}")
                        g8.append(gt)
                    for f16 in range(16):
                        mm = ps.tile([128, 512], F32, tag="mm", bufs=2,
                                     name=f"m1_{img}_{th}_{f16}")
                        for p in range(2):
                            nc.tensor.matmul(
                                mm,
                                _ap(wm1[p], 128 * f16, [[2048, 2], [1, 128]]),
                                _ap(z28, 2048 * p + 512 * th,
                                    [[1024, 2], [1, 512]]),
                                start=(p == 0), stop=(p == 1), perf_mode=DR)
                        nc.scalar.activation(
                            g8[f16 // 2][:, 512 * (f16 % 2):
                                         512 * (f16 % 2 + 1)],
                            mm, AF.Gelu, bias=d1_t[:, f16:f16 + 1],
                            scale=ISC)
                    for fo in range(4):
                        mm2 = ps.tile([128, 512], F32, tag="st", bufs=4,
                                      name=f"m2_{img}_{th}_{fo}")
                        for j in range(8):
                            nc.tensor.matmul(
                                mm2,
                                _ap(wm2[j], 128 * fo, [[512, 2], [1, 128]]),
                                _ap(g8[j], 0, [[512, 2], [1, 512]]),
                                start=(j == 0), stop=(j == 7), perf_mode=DR)
                        xv = xc[fo][:, 512 * th:512 * (th + 1)]
                        with tc.high_priority():
                            nc.vector.affine_then_add(
                                xout[fo][:, 512 * th:512 * (th + 1)],
                                mm2, xv, scale=ISC,
                                bias=b2_t[:, fo:fo + 1])

                for hf in range(2):
                    xor = sb.tile([128, 2048], BF16, tag="xraw", bufs=3,
                                  name=f"xor_{img}_{hf}")
                    for cc in range(2):
                        nc.vector.tensor_copy(
                            _ap(xor, 1024 * cc, RAS8),
                            _ap(xout[2 * hf + cc], 0, WIN8))
                    nc.gpsimd.dma_start(
                        out=bass.AP(
                            tensor=out_d.tensor,
                            offset=out_d.offset + img * C * H * W
                            + hf * 2 * 131072,
                            ap=[[1024, 128], [131072, 2], [1, 1024]]),
                        in_=xor)

            # ---------------- skewed pipeline ----------------
            st = {}
            st[0] = stage_a(0)
            if BI > 1:
                st[1] = stage_a(1)
            z28s = {}
            z28s[0] = stage_b(0, *st[0])
            for i in range(BI):
                if i + 2 < BI:
                    st[i + 2] = stage_a(i + 2)
                if i + 1 < BI:
                    z28s[i + 1] = stage_b(i + 1, *st[i + 1])
                stage_c(i, st[i][0], z28s[i])

    nc.compile()
    return nc


def _pair_pack(wT, nk_pairs):
    K, F = wT.shape
    assert K == nk_pairs * 256
    out = np.empty((nk_pairs, 128, 2 * F), dtype=E4)
    for p in range(nk_pairs):
        out[p, :, 0:F] = wT[256 * p:256 * p + 128, :].astype(E4)
        out[p, :, F:2 * F] = wT[256 * p + 128:256 * p + 256, :].astype(E4)
    return out


def _prep_weights(inputs):
    g1 = np.asarray(inputs["norm1_w"], np.float32)
    b1 = np.asarray(inputs["norm1_b"], np.float32)
    g2 = np.asarray(inputs["norm2_w"], np.float32)
    b2n = np.asarray(inputs["norm2_b"], np.float32)
    wqkv = np.array(inputs["qkv_w"], np.float32)
    bqkv = np.array(inputs["qkv_b"], np.float32)
    scale = HD ** -0.5
    wqkv[:C] *= scale
    bqkv = bqkv.copy()
    bqkv[:C] *= scale
    dqkv = wqkv @ b1 + bqkv
    wqkvT = (wqkv * g1[None, :]).T * WSC

    wproj = np.asarray(inputs["proj_w"], np.float32)
    dv = dqkv[2 * C:]
    pb = np.asarray(inputs["proj_b"], np.float32) + wproj @ dv
    wm1 = np.asarray(inputs["mlp_w1"], np.float32)
    d1 = wm1 @ b2n + np.asarray(inputs["mlp_b1"], np.float32)
    wm1T = (wm1 * g2[None, :]).T * WSC
    wm2 = np.asarray(inputs["mlp_w2"], np.float32)
    b2o = np.asarray(inputs["mlp_b2"], np.float32)

    rpb = np.asarray(inputs["rpb_table"], np.float32)
    rel = _relative_position_index(WS)
    bias = rpb[rel.reshape(-1)].reshape(N, N, NH)
    # additive score bias (pre-scaled by 64*64), off-window -> -inf-ish
    mask = np.full((128, NH, 128), -20.0 * WSC * WSC, np.float32)
    for wdx in range(8):
        mask[16 * wdx:16 * (wdx + 1), :, 16 * wdx:16 * (wdx + 1)] = \
            bias.transpose(1, 2, 0) * (WSC * WSC)
    # regroup into per-qt [128, 4heads*128q] tiles side by side
    m4 = mask.reshape(128, 4, 4, 128).transpose(1, 0, 2, 3)
    mask2d = np.ascontiguousarray(m4.reshape(4, 128, 512)
                                  .transpose(1, 0, 2).reshape(128, 2048))

    return {
        "wqkv": _pair_pack(np.ascontiguousarray(wqkvT), 2),
        "wproj": _pair_pack(np.ascontiguousarray(wproj.T * WSC), 2),
        "wm1": _pair_pack(np.ascontiguousarray(wm1T), 2),
        "wm2": _pair_pack(np.ascontiguousarray(wm2.T * WSC), 8),
        "dq64": np.ascontiguousarray(
            (WSC * dqkv[:C]).reshape(4, 128).T).astype(np.float32),
        "pb": np.ascontiguousarray(pb.reshape(4, 128).T).astype(np.float32),
        "d1": np.ascontiguousarray(d1.reshape(16, 128).T).astype(np.float32),
        "b2": np.ascontiguousarray(b2o.reshape(4, 128).T).astype(np.float32),
        "mask": mask2d.astype(BF),
        "idm": np.eye(128, dtype=BF),
    }


def get_program():
    if "nc" not in _cache:
        _cache["nc"] = _build_program()
    return _cache["nc"]


def make_in_maps(inputs):
    wmaps = _prep_weights(inputs)
    x_full = np.asarray(inputs["x"], np.float32)
    in_maps = []
    for core in range(NCORES):
        m = dict(wmaps)
        m["x"] = np.ascontiguousarray(x_full[BI * core:BI * (core + 1)])
        in_maps.append(m)
    return in_maps


def kernel(**inputs):
    nc = get_program()
    in_maps = make_in_maps(inputs)
    res = run_bass_kernel_spmd(nc, in_maps, list(range(NCORES)))
    out = np.concatenate([res.results[c]["out"] for c in range(NCORES)],
                         axis=0)
    return out


# revision 9
# speedup vs baseline: 1.0080x; 1.0080x over previous
"""Swin-style windowed-attention block (LN->W-MSA->residual->LN->MLP->residual)
for TRN2, data-parallel over batch across 8 NeuronCores.

Design (vs. the f32/bf16 channel-major baseline):
- 3-stage skewed software pipeline (A=load/LN1/qkv, B=attention/proj/LN2,
  C=MLP/store) emitted A0 A1 B0 | A2 B1 C0 | A3 B2 C1 | B3 C2 | C3 so the
  ACT engine's exp and gelu streams stay back-to-back across images.
- fp8e4m3 DoubleRow matmuls (2 k-tiles per instruction, 0.5 cyc/row) for
  qkv/v/proj/mlp1/mlp2; weights scaled x64 host-side into DoubleRow pair
  tiles; descales folded into activation scale slots (exp scale 1/4096
  absorbs the q64*k64 scale, gelu scale 1/64, affine_then_add scale 1/64).
- bf16 residual stream (4x DVE rate on the LN square/z passes); fp8 q/k
  tiles (halves SBUF + DMA traffic for the qh/kh head rearrange).
- relative-position bias + window mask folded into the score PSUM via one
  identity-stationary matmul per tile (off-window = -20 -> exp ~ 0), so
  softmax is exp-only on ACT with no mask multiply on DVE.
- k-bias dropped (cancels in softmax over keys); v-bias folded into the
  proj bias (softmax weights sum to 1); LN gamma/beta folded into the
  following matmul weights/biases.
- LayerNorm: ones-matmul stats on PE, Square/Sqrt+reciprocal soup (one
  act-table set; Gelu/Exp/Sqrt sets cost 1.28us per switch), r/mur rows
  broadcast across partitions via gpsimd partition_broadcast (input must
  be on partition 0 - the Q7 kernel ignores the AP base partition).
- residual updates via the fused affine_then_add custom DVE op; window
  permutation via 4-dim strided access patterns on DVE copies; casting
  DMAs (f32->bf16 loads, bf16->fp8 z/gelu quantization, bf16->f32 store)
  batched on the gpsimd SWDGE queue.
Note: TensorScalarPtr/TensorTensor ops do NOT lower for the Pool engine
(walrus ISA check) even though CoreSim accepts them - Pool runs only DMAs,
partition_broadcast and real ISA library ops.
"""
import sys
import numpy as np
import ml_dtypes

sys.path.insert(0, "/opt/trn_rl_repo")

import concourse.bass as bass
import concourse.bacc as bacc
import concourse.tile as tile
from concourse import mybir
from concourse.bass_utils import run_bass_kernel_spmd

F32 = mybir.dt.float32
BF16 = mybir.dt.bfloat16
FP8 = mybir.dt.float8e4
AF = mybir.ActivationFunctionType
ALU = mybir.AluOpType
DR = mybir.MatmulPerfMode.DoubleRow
BF = ml_dtypes.bfloat16
E4 = ml_dtypes.float8_e4m3

B, C, H, W = 32, 512, 32, 32
NH, WS = 16, 4
HD = C // NH
N = WS * WS
EPS = 1e-5
MLP_H = 4 * C
NCORES = 8
BI = B // NCORES
T = H * W

WSC = 64.0
ISC = 1.0 / WSC

_cache = {}


def _relative_position_index(ws):
    coords = np.stack(np.meshgrid(np.arange(ws), np.arange(ws), indexing="ij"))
    cf = coords.reshape(2, -1)
    rel = cf[:, :, None] - cf[:, None, :]
    rel = rel.transpose(1, 2, 0).astype(np.int64)
    rel[:, :, 0] += ws - 1
    rel[:, :, 1] += ws - 1
    rel[:, :, 0] *= 2 * ws - 1
    return rel.sum(-1)


def _ap(t, off, dims):
    return bass.AP(tensor=t.tensor, offset=t.offset + off,
                   ap=[t.ap[0]] + [list(d) for d in dims])


def _bc(t, n):
    return bass.AP(tensor=t.tensor, offset=t.offset,
                   ap=[list(t.ap[0]), [0, n]])


def _build_program():
    nc = bacc.Bacc("TRN2", target_bir_lowering=False, debug=False,
                   enable_asserts=True, num_devices=NCORES)

    def din(name, shape, dt):
        return nc.dram_tensor(name, shape, dt, kind="ExternalInput").ap()

    x_d = din("x", (BI, C, H, W), F32)
    wqkv_d = din("wqkv", (2, 128, 2 * 3 * C), FP8)
    wproj_d = din("wproj", (2, 128, 2 * C), FP8)
    wm1_d = din("wm1", (2, 128, 2 * MLP_H), FP8)
    wm2_d = din("wm2", (8, 128, 2 * C), FP8)
    dq64_d = din("dq64", (128, 4), F32)
    pb_d = din("pb", (128, 4), F32)
    d1_d = din("d1", (128, 16), F32)
    b2_d = din("b2", (128, 4), F32)
    mask_d = din("mask", (128, 4 * 512), BF16)
    idm_d = din("idm", (128, 128), BF16)

    out_d = nc.dram_tensor("out", (BI, C, H, W), F32,
                           kind="ExternalOutput").ap()

    with tile.TileContext(nc) as tc:
        with tc.tile_pool(name="sb", bufs=1) as sb, \
             tc.tile_pool(name="ps", bufs=1, space="PSUM") as ps:

            # ---------------- resident constants ----------------
            wqkv = []
            for p in range(2):
                t = sb.tile([128, 2 * 3 * C], FP8, tag="wqkv", bufs=2,
                            name=f"wqkv{p}")
                nc.sync.dma_start(out=t, in_=wqkv_d[p])
                wqkv.append(t)
            wproj = []
            for p in range(2):
                t = sb.tile([128, 2 * C], FP8, tag="wproj", bufs=2,
                            name=f"wproj{p}")
                nc.sync.dma_start(out=t, in_=wproj_d[p])
                wproj.append(t)
            wm1 = []
            for p in range(2):
                t = sb.tile([128, 2 * MLP_H], FP8, tag="wm1", bufs=2,
                            name=f"wm1{p}")
                nc.sync.dma_start(out=t, in_=wm1_d[p])
                wm1.append(t)
            wm2 = []
            for p in range(8):
                t = sb.tile([128, 2 * C], FP8, tag="wm2", bufs=8,
                            name=f"wm2{p}")
                nc.sync.dma_start(out=t, in_=wm2_d[p])
                wm2.append(t)
            mask_t = sb.tile([128, 4 * 512], BF16, tag="mask", name="mask_t")
            nc.sync.dma_start(out=mask_t, in_=mask_d)
            idm = sb.tile([128, 128], BF16, tag="idm", name="idm")
            nc.sync.dma_start(out=idm, in_=idm_d)
            dq_t = sb.tile([128, 4], F32, tag="dq", name="dq_t")
            nc.sync.dma_start(out=dq_t, in_=dq64_d)
            pb_t = sb.tile([128, 4], F32, tag="pbt", name="pb_t")
            nc.sync.dma_start(out=pb_t, in_=pb_d)
            d1_t = sb.tile([128, 16], F32, tag="d1t", name="d1_t")
            nc.sync.dma_start(out=d1_t, in_=d1_d)
            b2_t = sb.tile([128, 4], F32, tag="b2t", name="b2_t")
            nc.sync.dma_start(out=b2_t, in_=b2_d)
            onesb = sb.tile([128, 1], BF16, tag="onesb", name="onesb")
            nc.vector.memset(onesb, 1.0)
            onesf = sb.tile([128, 1], F32, tag="onesf", name="onesf")
            nc.vector.memset(onesf, 1.0)
            epsb = sb.tile([1, 1], F32, tag="epsb", name="epsb")
            nc.vector.memset(epsb, EPS)

            WIN8 = [[128, 8], [16, 8], [4, 4], [1, 4]]
            RAS8 = [[128, 8], [4, 8], [32, 4], [1, 4]]

            def layernorm(xc, sfx):
                rowsb = sb.tile([1, 1024], BF16, tag="rowsb", bufs=2,
                                name=f"rowsb_{sfx}")
                rowsm = sb.tile([1, 1024], BF16, tag="rowsm", bufs=2,
                                name=f"rowsm_{sfx}")
                x2 = {}
                for hh in range(2):
                    for c4 in range(4):
                        x2t = sb.tile([128, 512], BF16, tag="x2", bufs=8,
                                      name=f"x2_{sfx}_{hh}_{c4}")
                        nc.vector.tensor_mul(
                            x2t, xc[c4][:, 512 * hh:512 * (hh + 1)],
                            xc[c4][:, 512 * hh:512 * (hh + 1)])
                        x2[(hh, c4)] = x2t
                for hh in range(2):
                    sx = ps.tile([1, 512], F32, tag="av", bufs=2,
                                 name=f"sx_{sfx}_{hh}")
                    sx2 = ps.tile([1, 512], F32, tag="av", bufs=2,
                                  name=f"sx2_{sfx}_{hh}")
                    sl0 = slice(512 * hh, 512 * (hh + 1))
                    for c4 in range(4):
                        nc.tensor.matmul(sx[0:1, :], onesb,
                                         xc[c4][:, sl0],
                                         start=(c4 == 0), stop=(c4 == 3))
                    for c4 in range(4):
                        nc.tensor.matmul(sx2[0:1, :], onesb, x2[(hh, c4)],
                                         start=(c4 == 0), stop=(c4 == 3))
                    sl = slice(512 * hh, 512 * (hh + 1))
                    with tc.high_priority():
                        mu2 = sb.tile([1, 512], F32, tag="tsq", bufs=2,
                                      name=f"mu2_{sfx}_{hh}")
                        nc.scalar.activation(mu2, sx[0:1, :], AF.Square,
                                             scale=1.0 / C)
                        varc = sb.tile([1, 512], F32, tag="varc", bufs=2,
                                       name=f"varc_{sfx}_{hh}")
                        nc.vector.scalar_tensor_tensor(
                            varc, sx2[0:1, :], 1.0 / C, mu2,
                            ALU.mult, ALU.subtract)
                        sdev = sb.tile([1, 512], F32, tag="rln", bufs=2,
                                       name=f"sdev_{sfx}_{hh}")
                        nc.scalar.activation(sdev, varc, AF.Sqrt, bias=epsb)
                        with nc.allow_low_precision(
                                reason="bf16 r/mur rows are intentional"):
                            nc.vector.reciprocal(rowsb[0:1, sl], sdev)
                            nc.vector.scalar_tensor_tensor(
                                rowsm[0:1, sl], sx[0:1, :], 1.0 / C,
                                rowsb[0:1, sl], ALU.mult, ALU.mult)
                rm_bc = sb.tile([128, 2048], BF16, tag="rmbc", bufs=2,
                                name=f"rmbc_{sfx}")
                nc.gpsimd.partition_broadcast(rm_bc[:, 0:1024],
                                              rowsb[0:1, :], channels=128)
                nc.gpsimd.partition_broadcast(rm_bc[:, 1024:2048],
                                              rowsm[0:1, :], channels=128)
                return rm_bc[:, 0:1024], rm_bc[:, 1024:2048]

            def z_quant(xc, r_bc, mur_bc, sfx):
                z8 = sb.tile([128, 4096], FP8, tag="z8p", bufs=3,
                             name=f"z8_{sfx}")
                for hf in range(2):
                    zb = sb.tile([128, 2048], BF16, tag="zb", bufs=2,
                                 name=f"zb_{sfx}_{hf}")
                    for cc in range(2):
                        c4 = 2 * hf + cc
                        t1 = sb.tile([128, 1024], BF16, tag="zt1", bufs=1,
                                     name=f"t1_{sfx}_{c4}")
                        nc.vector.tensor_mul(t1, xc[c4], r_bc)
                        nc.vector.tensor_tensor(
                            out=zb[:, 1024 * cc:1024 * (cc + 1)],
                            in0=t1, in1=mur_bc, op=ALU.subtract)
                    nc.gpsimd.dma_start(
                        out=z8[:, 2048 * hf:2048 * (hf + 1)], in_=zb)
                return z8

            # ---------------- stage A: load + LN1 + qkv ----------------
            def stage_a(img):
                xc = []
                for hf in range(2):
                    xrw = sb.tile([128, 2048], BF16, tag="xraw", bufs=3,
                                  name=f"xr_{img}_{hf}")
                    nc.gpsimd.dma_start(
                        out=xrw,
                        in_=bass.AP(
                            tensor=x_d.tensor,
                            offset=x_d.offset + img * C * H * W
                            + hf * 2 * 131072,
                            ap=[[1024, 128], [131072, 2], [1, 1024]]))
                    for cc in range(2):
                        c4 = 2 * hf + cc
                        xt = sb.tile([128, 1024], BF16, tag="xc", bufs=12,
                                     name=f"x_{img}_{c4}")
                        nc.vector.tensor_copy(_ap(xt, 0, WIN8),
                                              _ap(xrw, 1024 * cc, RAS8))
                        xc.append(xt)
                with tc.high_priority():
                    r_bc, mur_bc = layernorm(xc, f"l1_{img}")
                z8 = z_quant(xc, r_bc, mur_bc, f"l1_{img}")

                # qkv: q/k fp8 f-tiles (channel-major)
                qk = {}
                for fi in (0, 4, 1, 5, 2, 6, 3, 7):
                    qkt = sb.tile([128, 1024], FP8, tag="qk", bufs=12,
                                  name=f"qk_{img}_{fi}")
                    for th in range(2):
                        mm = ps.tile([128, 512], F32, tag="mm", bufs=2,
                                     name=f"qkp_{img}_{fi}_{th}")
                        for p in range(2):
                            nc.tensor.matmul(
                                mm,
                                _ap(wqkv[p], 128 * fi, [[1536, 2], [1, 128]]),
                                _ap(z8, 2048 * p + 512 * th,
                                    [[1024, 2], [1, 512]]),
                                start=(p == 0), stop=(p == 1), perf_mode=DR)
                        with tc.high_priority():
                            if fi < 4:
                                nc.scalar.activation(
                                    qkt[:, 512 * th:512 * (th + 1)], mm,
                                    AF.Identity, bias=dq_t[:, fi:fi + 1])
                            else:
                                nc.vector.tensor_copy(
                                    qkt[:, 512 * th:512 * (th + 1)], mm)
                    qk[fi] = qkt

                # v (token-major bf16 + interleaved ones column)
                vaug = []
                for g in range(8):
                    mm = ps.tile([128, 512], F32, tag="mm", bufs=2,
                                 name=f"vp_{img}_{g}")
                    for p in range(2):
                        nc.tensor.matmul(
                            mm,
                            _ap(z8, 2048 * p + 128 * g, [[1024, 2], [1, 128]]),
                            _ap(wqkv[p], 2 * C, [[1536, 2], [1, 512]]),
                            start=(p == 0), stop=(p == 1), perf_mode=DR)
                    va = sb.tile([128, 33 * NH], BF16, tag="vaug", bufs=16,
                                 name=f"va_{img}_{g}")
                    nc.vector.memset(_ap(va, 32, [[33, NH]]), 1.0)
                    nc.scalar.mul(
                        _ap(va, 0, [[33, NH], [1, 32]]),
                        _ap(mm, 0, [[32, NH], [1, 32]]), ISC)
                    vaug.append(va)
                return xc, qk, vaug

            # ------- stage B: attention + proj + LN2 + z28 -------
            def stage_b(img, xc, qk, vaug):
                atc = [sb.tile([128, 512], BF16, tag="atc", bufs=8,
                               name=f"atc_{img}_{g}") for g in range(8)]
                for qt in range(4):
                    qh = sb.tile([32, 4096], FP8, tag="qh", bufs=3,
                                 name=f"qh_{img}_{qt}")
                    kh = sb.tile([32, 4096], FP8, tag="kh", bufs=3,
                                 name=f"kh_{img}_{qt}")
                    for b4 in range(4):
                        nc.sync.dma_start(
                            out=qh[0:32, 1024 * b4:1024 * (b4 + 1)],
                            in_=qk[qt][32 * b4:32 * (b4 + 1), :])
                        nc.sync.dma_start(
                            out=kh[0:32, 1024 * b4:1024 * (b4 + 1)],
                            in_=qk[4 + qt][32 * b4:32 * (b4 + 1), :])
                    for g in range(8):
                        stp = ps.tile([128, 512], F32, tag="st", bufs=4,
                                      name=f"stp_{img}_{qt}_{g}")
                        nc.tensor.matmul(
                            stp, idm, mask_t[:, 512 * qt:512 * (qt + 1)],
                            start=True, stop=False, skip_group_check=True)
                        for b4 in range(4):
                            sl = slice(1024 * b4 + 128 * g,
                                       1024 * b4 + 128 * (g + 1))
                            nc.tensor.matmul(
                                stp[:, 128 * b4:128 * (b4 + 1)],
                                kh[0:32, sl], qh[0:32, sl],
                                start=False, stop=(b4 == 3),
                                skip_group_check=True)
                        pt = sb.tile([128, 512], BF16, tag="pt", bufs=4,
                                     name=f"pt_{img}_{qt}_{g}")
                        nc.scalar.activation(pt, stp, AF.Exp,
                                             scale=1.0 / (WSC * WSC))
                        av = ps.tile([128, 132], F32, tag="av", bufs=2,
                                     name=f"av_{img}_{qt}_{g}")
                        for b4 in range(4):
                            h = 4 * qt + b4
                            nc.tensor.matmul(
                                av[:, 33 * b4:33 * (b4 + 1)],
                                pt[:, 128 * b4:128 * (b4 + 1)],
                                vaug[g][:, 33 * h:33 * (h + 1)],
                                start=True, stop=True)
                        rec = sb.tile([128, 4], F32, tag="rec", bufs=8,
                                      name=f"rec_{img}_{qt}_{g}")
                        nc.vector.reciprocal(rec, _ap(av, 32, [[33, 4]]))
                        nc.vector.tensor_tensor(
                            out=_ap(atc[g], 128 * qt, [[32, 4], [1, 32]]),
                            in0=_ap(av, 0, [[33, 4], [1, 32]]),
                            in1=_ap(rec, 0, [[1, 4], [0, 32]]),
                            op=ALU.mult)

                actn8 = [sb.tile([128, 2048], FP8, tag="actn", bufs=2,
                                 name=f"actn_{img}_{p}") for p in range(2)]
                for fp in range(4):
                    for Q in range(2):
                        tp = ps.tile([128, 512], BF16, tag="mm", bufs=2,
                                     name=f"tp_{img}_{fp}_{Q}")
                        for gq in range(4):
                            g = 4 * Q + gq
                            nc.tensor.transpose(
                                tp[:, 128 * gq:128 * (gq + 1)],
                                atc[g][:, 128 * fp:128 * (fp + 1)], idm)
                        dst = actn8[fp // 2][:, 1024 * (fp % 2) + 512 * Q:
                                             1024 * (fp % 2) + 512 * (Q + 1)]
                        if fp % 2 == 0:
                            nc.vector.tensor_copy(dst, tp)
                        else:
                            nc.scalar.copy(dst, tp)

                for th in range(2):
                    for fo in range(4):
                        mm = ps.tile([128, 512], F32, tag="mm", bufs=2,
                                     name=f"pj_{img}_{fo}_{th}")
                        for p in range(2):
                            nc.tensor.matmul(
                                mm,
                                _ap(wproj[p], 128 * fo, [[512, 2], [1, 128]]),
                                _ap(actn8[p], 512 * th, [[1024, 2], [1, 512]]),
                                start=(p == 0), stop=(p == 1), perf_mode=DR)
                        xv = xc[fo][:, 512 * th:512 * (th + 1)]
                        nc.vector.affine_then_add(
                            xv, mm, xv, scale=ISC,
                            bias=pb_t[:, fo:fo + 1])

                with tc.high_priority():
                    r2_bc, mur2_bc = layernorm(xc, f"l2_{img}")
                z28 = z_quant(xc, r2_bc, mur2_bc, f"l2_{img}")
                return z28

            # ---------------- stage C: MLP + store ----------------
            def stage_c(img, xc, z28):
                xout = [sb.tile([128, 1024], BF16, tag="xout", bufs=4,
                                name=f"xo_{img}_{c4}") for c4 in range(4)]
                for th in range(2):
                    g8 = []
                    for j in range(8):
                        gt = sb.tile([128, 1024], FP8, tag="g8", bufs=8,
                                     name=f"g8_{img}_{th}_{j}")
                        g8.append(gt)
                    for f16 in range(16):
                        mm = ps.tile([128, 512], F32, tag="mm", bufs=2,
                                     name=f"m1_{img}_{th}_{f16}")
                        for p in range(2):
                            nc.tensor.matmul(
                                mm,
                                _ap(wm1[p], 128 * f16, [[2048, 2], [1, 128]]),
                                _ap(z28, 2048 * p + 512 * th,
                                    [[1024, 2], [1, 512]]),
                                start=(p == 0), stop=(p == 1), perf_mode=DR)
                        nc.scalar.activation(
                            g8[f16 // 2][:, 512 * (f16 % 2):
                                         512 * (f16 % 2 + 1)],
                            mm, AF.Gelu, bias=d1_t[:, f16:f16 + 1],
                            scale=ISC)
                    for fo in range(4):
                        mm2 = ps.tile([128, 512], F32, tag="st", bufs=4,
                                      name=f"m2_{img}_{th}_{fo}")
                        for j in range(8):
                            nc.tensor.matmul(
                                mm2,
                                _ap(wm2[j], 128 * fo, [[512, 2], [1, 128]]),
                                _ap(g8[j], 0, [[512, 2], [1, 512]]),
                                start=(j == 0), stop=(j == 7), perf_mode=DR)
                        xv = xc[fo][:, 512 * th:512 * (th + 1)]
                        with tc.high_priority():
                            nc.vector.affine_then_add(
                                xout[fo][:, 512 * th:512 * (th + 1)],
                                mm2, xv, scale=ISC,
                                bias=b2_t[:, fo:fo + 1])

                for hf in range(2):
                    xor = sb.tile([128, 2048], BF16, tag="xraw", bufs=3,
                                  name=f"xor_{img}_{hf}")
                    for cc in range(2):
                        nc.vector.tensor_copy(
                            _ap(xor, 1024 * cc, RAS8),
                            _ap(xout[2 * hf + cc], 0, WIN8))
                    nc.gpsimd.dma_start(
                        out=bass.AP(
                            tensor=out_d.tensor,
                            offset=out_d.offset + img * C * H * W
                            + hf * 2 * 131072,
                            ap=[[1024, 128], [131072, 2], [1, 1024]]),
                        in_=xor)

            # ---------------- skewed pipeline ----------------
            st = {}
            st[0] = stage_a(0)
            if BI > 1:
                st[1] = stage_a(1)
            z28s = {}
            z28s[0] = stage_b(0, *st[0])
            for i in range(BI):
                if i + 2 < BI:
                    st[i + 2] = stage_a(i + 2)
                if i + 1 < BI:
                    z28s[i + 1] = stage_b(i + 1, *st[i + 1])
                stage_c(i, st[i][0], z28s[i])

    nc.compile()
    return nc


def _pair_pack(wT, nk_pairs):
    K, F = wT.shape
    assert K == nk_pairs * 256
    out = np.empty((nk_pairs, 128, 2 * F), dtype=E4)
    for p in range(nk_pairs):
        out[p, :, 0:F] = wT[256 * p:256 * p + 128, :].astype(E4)
        out[p, :, F:2 * F] = wT[256 * p + 128:256 * p + 256, :].astype(E4)
    return out


def _prep_weights(inputs):
    g1 = np.asarray(inputs["norm1_w"], np.float32)
    b1 = np.asarray(inputs["norm1_b"], np.float32)
    g2 = np.asarray(inputs["norm2_w"], np.float32)
    b2n = np.asarray(inputs["norm2_b"], np.float32)
    wqkv = np.array(inputs["qkv_w"], np.float32)
    bqkv = np.array(inputs["qkv_b"], np.float32)
    scale = HD ** -0.5
    wqkv[:C] *= scale
    bqkv = bqkv.copy()
    bqkv[:C] *= scale
    dqkv = wqkv @ b1 + bqkv
    wqkvT = (wqkv * g1[None, :]).T * WSC

    wproj = np.asarray(inputs["proj_w"], np.float32)
    dv = dqkv[2 * C:]
    pb = np.asarray(inputs["proj_b"], np.float32) + wproj @ dv
    wm1 = np.asarray(inputs["mlp_w1"], np.float32)
    d1 = wm1 @ b2n + np.asarray(inputs["mlp_b1"], np.float32)
    wm1T = (wm1 * g2[None, :]).T * WSC
    wm2 = np.asarray(inputs["mlp_w2"], np.float32)
    b2o = np.asarray(inputs["mlp_b2"], np.float32)

    rpb = np.asarray(inputs["rpb_table"], np.float32)
    rel = _relative_position_index(WS)
    bias = rpb[rel.reshape(-1)].reshape(N, N, NH)
    # additive score bias (pre-scaled by 64*64), off-window -> -inf-ish
    mask = np.full((128, NH, 128), -20.0 * WSC * WSC, np.float32)
    for wdx in range(8):
        mask[16 * wdx:16 * (wdx + 1), :, 16 * wdx:16 * (wdx + 1)] = \
            bias.transpose(1, 2, 0) * (WSC * WSC)
    # regroup into per-qt [128, 4heads*128q] tiles side by side
    m4 = mask.reshape(128, 4, 4, 128).transpose(1, 0, 2, 3)
    mask2d = np.ascontiguousarray(m4.reshape(4, 128, 512)
                                  .transpose(1, 0, 2).reshape(128, 2048))

    return {
        "wqkv": _pair_pack(np.ascontiguousarray(wqkvT), 2),
        "wproj": _pair_pack(np.ascontiguousarray(wproj.T * WSC), 2),
        "wm1": _pair_pack(np.ascontiguousarray(wm1T), 2),
        "wm2": _pair_pack(np.ascontiguousarray(wm2.T * WSC), 8),
        "dq64": np.ascontiguousarray(
            (WSC * dqkv[:C]).reshape(4, 128).T).astype(np.float32),
        "pb": np.ascontiguousarray(pb.reshape(4, 128).T).astype(np.float32),
        "d1": np.ascontiguousarray(d1.reshape(16, 128).T).astype(np.float32),
        "b2": np.ascontiguousarray(b2o.reshape(4, 128).T).astype(np.float32),
        "mask": mask2d.astype(BF),
        "idm": np.eye(128, dtype=BF),
    }


def get_program():
    if "nc" not in _cache:
        _cache["nc"] = _build_program()
    return _cache["nc"]


def make_in_maps(inputs):
    wmaps = _prep_weights(inputs)
    x_full = np.asarray(inputs["x"], np.float32)
    in_maps = []
    for core in range(NCORES):
        m = dict(wmaps)
        m["x"] = np.ascontiguousarray(x_full[BI * core:BI * (core + 1)])
        in_maps.append(m)
    return in_maps


def kernel(**inputs):
    nc = get_program()
    in_maps = make_in_maps(inputs)
    res = run_bass_kernel_spmd(nc, in_maps, list(range(NCORES)))
    out = np.concatenate([res.results[c]["out"] for c in range(NCORES)],
                         axis=0)
    return out


# revision 10
# speedup vs baseline: 1.0105x; 1.0025x over previous
"""Swin-style windowed-attention block (LN->W-MSA->residual->LN->MLP->residual)
for TRN2, data-parallel over batch across 8 NeuronCores.

Design (vs. the f32/bf16 channel-major baseline):
- 3-stage skewed software pipeline (A=load/LN1/qkv, B=attention/proj/LN2,
  C=MLP/store) emitted A0 A1 B0 | A2 B1 C0 | A3 B2 C1 | B3 C2 | C3 so the
  ACT engine's exp and gelu streams stay back-to-back across images.
- fp8e4m3 DoubleRow matmuls (2 k-tiles per instruction, 0.5 cyc/row) for
  qkv/v/proj/mlp1/mlp2; weights scaled x64 host-side into DoubleRow pair
  tiles; descales folded into activation scale slots (exp scale 1/4096
  absorbs the q64*k64 scale, gelu scale 1/64, affine_then_add scale 1/64).
- bf16 residual stream (4x DVE rate on the LN square/z passes); fp8 q/k
  tiles (halves SBUF + DMA traffic for the qh/kh head rearrange).
- relative-position bias + window mask folded into the score PSUM via one
  identity-stationary matmul per tile (off-window = -20 -> exp ~ 0), so
  softmax is exp-only on ACT with no mask multiply on DVE.
- k-bias dropped (cancels in softmax over keys); v-bias folded into the
  proj bias (softmax weights sum to 1); LN gamma/beta folded into the
  following matmul weights/biases.
- LayerNorm: ones-matmul stats on PE, Square/Sqrt+reciprocal soup (one
  act-table set; Gelu/Exp/Sqrt sets cost 1.28us per switch), r/mur rows
  broadcast across partitions via gpsimd partition_broadcast (input must
  be on partition 0 - the Q7 kernel ignores the AP base partition).
- residual updates via the fused affine_then_add custom DVE op; window
  permutation via 4-dim strided access patterns on DVE copies; casting
  DMAs (f32->bf16 loads, bf16->fp8 z/gelu quantization, bf16->f32 store)
  batched on the gpsimd SWDGE queue.
Note: TensorScalarPtr/TensorTensor ops do NOT lower for the Pool engine
(walrus ISA check) even though CoreSim accepts them - Pool runs only DMAs,
partition_broadcast and real ISA library ops.
"""
import sys
import numpy as np
import ml_dtypes

sys.path.insert(0, "/opt/trn_rl_repo")

import concourse.bass as bass
import concourse.bacc as bacc
import concourse.tile as tile
from concourse import mybir
from concourse.bass_utils import run_bass_kernel_spmd

F32 = mybir.dt.float32
BF16 = mybir.dt.bfloat16
FP8 = mybir.dt.float8e4
AF = mybir.ActivationFunctionType
ALU = mybir.AluOpType
DR = mybir.MatmulPerfMode.DoubleRow
BF = ml_dtypes.bfloat16
E4 = ml_dtypes.float8_e4m3

B, C, H, W = 32, 512, 32, 32
NH, WS = 16, 4
HD = C // NH
N = WS * WS
EPS = 1e-5
MLP_H = 4 * C
NCORES = 8
BI = B // NCORES
T = H * W

WSC = 64.0
ISC = 1.0 / WSC

_cache = {}


def _relative_position_index(ws):
    coords = np.stack(np.meshgrid(np.arange(ws), np.arange(ws), indexing="ij"))
    cf = coords.reshape(2, -1)
    rel = cf[:, :, None] - cf[:, None, :]
    rel = rel.transpose(1, 2, 0).astype(np.int64)
    rel[:, :, 0] += ws - 1
    rel[:, :, 1] += ws - 1
    rel[:, :, 0] *= 2 * ws - 1
    return rel.sum(-1)


def _ap(t, off, dims):
    return bass.AP(tensor=t.tensor, offset=t.offset + off,
                   ap=[t.ap[0]] + [list(d) for d in dims])


def _bc(t, n):
    return bass.AP(tensor=t.tensor, offset=t.offset,
                   ap=[list(t.ap[0]), [0, n]])


def _build_program():
    nc = bacc.Bacc("TRN2", target_bir_lowering=False, debug=False,
                   enable_asserts=True, num_devices=NCORES)

    def din(name, shape, dt):
        return nc.dram_tensor(name, shape, dt, kind="ExternalInput").ap()

    x_d = din("x", (BI, C, H, W), F32)
    wqkv_d = din("wqkv", (2, 128, 2 * 3 * C), FP8)
    wproj_d = din("wproj", (2, 128, 2 * C), FP8)
    wm1_d = din("wm1", (2, 128, 2 * MLP_H), FP8)
    wm2_d = din("wm2", (8, 128, 2 * C), FP8)
    dq64_d = din("dq64", (128, 4), F32)
    pb_d = din("pb", (128, 4), F32)
    d1_d = din("d1", (128, 16), F32)
    b2_d = din("b2", (128, 4), F32)
    mask_d = din("mask", (128, 4 * 512), BF16)
    idm_d = din("idm", (128, 128), BF16)

    out_d = nc.dram_tensor("out", (BI, C, H, W), F32,
                           kind="ExternalOutput").ap()

    with tile.TileContext(nc) as tc:
        with tc.tile_pool(name="sb", bufs=1) as sb, \
             tc.tile_pool(name="ps", bufs=1, space="PSUM") as ps:

            # ---------------- resident constants ----------------
            wqkv = []
            for p in range(2):
                t = sb.tile([128, 2 * 3 * C], FP8, tag="wqkv", bufs=2,
                            name=f"wqkv{p}")
                nc.sync.dma_start(out=t, in_=wqkv_d[p])
                wqkv.append(t)
            wproj = []
            wm1 = []
            wm2 = []
            mask_holder = []

            def load_late_weights():
                for p in range(2):
                    t = sb.tile([128, 2 * C], FP8, tag="wproj", bufs=2,
                                name=f"wproj{p}")
                    nc.sync.dma_start(out=t, in_=wproj_d[p])
                    wproj.append(t)
                mask_t = sb.tile([128, 4 * 512], BF16, tag="mask",
                                 name="mask_t")
                nc.sync.dma_start(out=mask_t, in_=mask_d)
                mask_holder.append(mask_t)
                for p in range(2):
                    t = sb.tile([128, 2 * MLP_H], FP8, tag="wm1", bufs=2,
                                name=f"wm1{p}")
                    nc.sync.dma_start(out=t, in_=wm1_d[p])
                    wm1.append(t)
                for p in range(8):
                    t = sb.tile([128, 2 * C], FP8, tag="wm2", bufs=8,
                                name=f"wm2{p}")
                    nc.sync.dma_start(out=t, in_=wm2_d[p])
                    wm2.append(t)
            idm = sb.tile([128, 128], BF16, tag="idm", name="idm")
            nc.sync.dma_start(out=idm, in_=idm_d)
            dq_t = sb.tile([128, 4], F32, tag="dq", name="dq_t")
            nc.sync.dma_start(out=dq_t, in_=dq64_d)
            pb_t = sb.tile([128, 4], F32, tag="pbt", name="pb_t")
            nc.sync.dma_start(out=pb_t, in_=pb_d)
            d1_t = sb.tile([128, 16], F32, tag="d1t", name="d1_t")
            nc.sync.dma_start(out=d1_t, in_=d1_d)
            b2_t = sb.tile([128, 4], F32, tag="b2t", name="b2_t")
            nc.sync.dma_start(out=b2_t, in_=b2_d)
            onesb = sb.tile([128, 1], BF16, tag="onesb", name="onesb")
            nc.vector.memset(onesb, 1.0)
            onesf = sb.tile([128, 1], F32, tag="onesf", name="onesf")
            nc.vector.memset(onesf, 1.0)
            epsb = sb.tile([1, 1], F32, tag="epsb", name="epsb")
            nc.vector.memset(epsb, EPS)

            WIN8 = [[128, 8], [16, 8], [4, 4], [1, 4]]
            RAS8 = [[128, 8], [4, 8], [32, 4], [1, 4]]

            def layernorm(xc, sfx):
                rowsb = sb.tile([1, 1024], BF16, tag="rowsb", bufs=2,
                                name=f"rowsb_{sfx}")
                rowsm = sb.tile([1, 1024], BF16, tag="rowsm", bufs=2,
                                name=f"rowsm_{sfx}")
                x2 = {}
                for hh in range(2):
                    for c4 in range(4):
                        x2t = sb.tile([128, 512], BF16, tag="x2", bufs=8,
                                      name=f"x2_{sfx}_{hh}_{c4}")
                        nc.vector.tensor_mul(
                            x2t, xc[c4][:, 512 * hh:512 * (hh + 1)],
                            xc[c4][:, 512 * hh:512 * (hh + 1)])
                        x2[(hh, c4)] = x2t
                for hh in range(2):
                    sx = ps.tile([1, 512], F32, tag="av", bufs=2,
                                 name=f"sx_{sfx}_{hh}")
                    sx2 = ps.tile([1, 512], F32, tag="av", bufs=2,
                                  name=f"sx2_{sfx}_{hh}")
                    sl0 = slice(512 * hh, 512 * (hh + 1))
                    for c4 in range(4):
                        nc.tensor.matmul(sx[0:1, :], onesb,
                                         xc[c4][:, sl0],
                                         start=(c4 == 0), stop=(c4 == 3))
                    for c4 in range(4):
                        nc.tensor.matmul(sx2[0:1, :], onesb, x2[(hh, c4)],
                                         start=(c4 == 0), stop=(c4 == 3))
                    sl = slice(512 * hh, 512 * (hh + 1))
                    with tc.high_priority():
                        mu2 = sb.tile([1, 512], F32, tag="tsq", bufs=2,
                                      name=f"mu2_{sfx}_{hh}")
                        nc.scalar.activation(mu2, sx[0:1, :], AF.Square,
                                             scale=1.0 / C)
                        varc = sb.tile([1, 512], F32, tag="varc", bufs=2,
                                       name=f"varc_{sfx}_{hh}")
                        nc.vector.scalar_tensor_tensor(
                            varc, sx2[0:1, :], 1.0 / C, mu2,
                            ALU.mult, ALU.subtract)
                        sdev = sb.tile([1, 512], F32, tag="rln", bufs=2,
                                       name=f"sdev_{sfx}_{hh}")
                        nc.scalar.activation(sdev, varc, AF.Sqrt, bias=epsb)
                        with nc.allow_low_precision(
                                reason="bf16 r/mur rows are intentional"):
                            nc.vector.reciprocal(rowsb[0:1, sl], sdev)
                            nc.vector.scalar_tensor_tensor(
                                rowsm[0:1, sl], sx[0:1, :], 1.0 / C,
                                rowsb[0:1, sl], ALU.mult, ALU.mult)
                rm_bc = sb.tile([128, 2048], BF16, tag="rmbc", bufs=2,
                                name=f"rmbc_{sfx}")
                nc.gpsimd.partition_broadcast(rm_bc[:, 0:1024],
                                              rowsb[0:1, :], channels=128)
                nc.gpsimd.partition_broadcast(rm_bc[:, 1024:2048],
                                              rowsm[0:1, :], channels=128)
                return rm_bc[:, 0:1024], rm_bc[:, 1024:2048]

            def z_quant(xc, r_bc, mur_bc, sfx):
                z8 = sb.tile([128, 4096], FP8, tag="z8p", bufs=3,
                             name=f"z8_{sfx}")
                for hf in range(2):
                    zb = sb.tile([128, 2048], BF16, tag="zb", bufs=2,
                                 name=f"zb_{sfx}_{hf}")
                    for cc in range(2):
                        c4 = 2 * hf + cc
                        t1 = sb.tile([128, 1024], BF16, tag="zt1", bufs=1,
                                     name=f"t1_{sfx}_{c4}")
                        nc.vector.tensor_mul(t1, xc[c4], r_bc)
                        nc.vector.tensor_tensor(
                            out=zb[:, 1024 * cc:1024 * (cc + 1)],
                            in0=t1, in1=mur_bc, op=ALU.subtract)
                    nc.gpsimd.dma_start(
                        out=z8[:, 2048 * hf:2048 * (hf + 1)], in_=zb)
                return z8

            # ---------------- stage A: load + LN1 + qkv ----------------
            def stage_a(img):
                xc = []
                for hf in range(2):
                    xrw = sb.tile([128, 2048], BF16, tag="xraw", bufs=3,
                                  name=f"xr_{img}_{hf}")
                    nc.gpsimd.dma_start(
                        out=xrw,
                        in_=bass.AP(
                            tensor=x_d.tensor,
                            offset=x_d.offset + img * C * H * W
                            + hf * 2 * 131072,
                            ap=[[1024, 128], [131072, 2], [1, 1024]]))
                    for cc in range(2):
                        c4 = 2 * hf + cc
                        xt = sb.tile([128, 1024], BF16, tag="xc", bufs=12,
                                     name=f"x_{img}_{c4}")
                        nc.vector.tensor_copy(_ap(xt, 0, WIN8),
                                              _ap(xrw, 1024 * cc, RAS8))
                        xc.append(xt)
                with tc.high_priority():
                    r_bc, mur_bc = layernorm(xc, f"l1_{img}")
                z8 = z_quant(xc, r_bc, mur_bc, f"l1_{img}")

                # qkv: q/k fp8 f-tiles (channel-major)
                qk = {}
                for fi in (0, 4, 1, 5, 2, 6, 3, 7):
                    qkt = sb.tile([128, 1024], FP8, tag="qk", bufs=12,
                                  name=f"qk_{img}_{fi}")
                    for th in range(2):
                        mm = ps.tile([128, 512], F32, tag="mm", bufs=2,
                                     name=f"qkp_{img}_{fi}_{th}")
                        for p in range(2):
                            nc.tensor.matmul(
                                mm,
                                _ap(wqkv[p], 128 * fi, [[1536, 2], [1, 128]]),
                                _ap(z8, 2048 * p + 512 * th,
                                    [[1024, 2], [1, 512]]),
                                start=(p == 0), stop=(p == 1), perf_mode=DR)
                        with tc.high_priority():
                            if fi < 4:
                                nc.scalar.activation(
                                    qkt[:, 512 * th:512 * (th + 1)], mm,
                                    AF.Identity, bias=dq_t[:, fi:fi + 1])
                            else:
                                nc.vector.tensor_copy(
                                    qkt[:, 512 * th:512 * (th + 1)], mm)
                    qk[fi] = qkt

                # v (token-major bf16 + interleaved ones column)
                vaug = []
                for g in range(8):
                    mm = ps.tile([128, 512], F32, tag="mm", bufs=2,
                                 name=f"vp_{img}_{g}")
                    for p in range(2):
                        nc.tensor.matmul(
                            mm,
                            _ap(z8, 2048 * p + 128 * g, [[1024, 2], [1, 128]]),
                            _ap(wqkv[p], 2 * C, [[1536, 2], [1, 512]]),
                            start=(p == 0), stop=(p == 1), perf_mode=DR)
                    va = sb.tile([128, 33 * NH], BF16, tag="vaug", bufs=16,
                                 name=f"va_{img}_{g}")
                    nc.vector.memset(_ap(va, 32, [[33, NH]]), 1.0)
                    nc.scalar.mul(
                        _ap(va, 0, [[33, NH], [1, 32]]),
                        _ap(mm, 0, [[32, NH], [1, 32]]), ISC)
                    vaug.append(va)
                return xc, qk, vaug

            # ------- stage B: attention + proj + LN2 + z28 -------
            def stage_b(img, xc, qk, vaug):
                atc = [sb.tile([128, 512], BF16, tag="atc", bufs=8,
                               name=f"atc_{img}_{g}") for g in range(8)]
                for qt in range(4):
                    qh = sb.tile([32, 4096], FP8, tag="qh", bufs=3,
                                 name=f"qh_{img}_{qt}")
                    kh = sb.tile([32, 4096], FP8, tag="kh", bufs=3,
                                 name=f"kh_{img}_{qt}")
                    for b4 in range(4):
                        nc.sync.dma_start(
                            out=qh[0:32, 1024 * b4:1024 * (b4 + 1)],
                            in_=qk[qt][32 * b4:32 * (b4 + 1), :])
                        nc.sync.dma_start(
                            out=kh[0:32, 1024 * b4:1024 * (b4 + 1)],
                            in_=qk[4 + qt][32 * b4:32 * (b4 + 1), :])
                    for g in range(8):
                        stp = ps.tile([128, 512], F32, tag="st", bufs=4,
                                      name=f"stp_{img}_{qt}_{g}")
                        nc.tensor.matmul(
                            stp, idm,
                            mask_holder[0][:, 512 * qt:512 * (qt + 1)],
                            start=True, stop=False, skip_group_check=True)
                        for b4 in range(4):
                            sl = slice(1024 * b4 + 128 * g,
                                       1024 * b4 + 128 * (g + 1))
                            nc.tensor.matmul(
                                stp[:, 128 * b4:128 * (b4 + 1)],
                                kh[0:32, sl], qh[0:32, sl],
                                start=False, stop=(b4 == 3),
                                skip_group_check=True)
                        pt = sb.tile([128, 512], BF16, tag="pt", bufs=4,
                                     name=f"pt_{img}_{qt}_{g}")
                        nc.scalar.activation(pt, stp, AF.Exp,
                                             scale=1.0 / (WSC * WSC))
                        av = ps.tile([128, 132], F32, tag="av", bufs=2,
                                     name=f"av_{img}_{qt}_{g}")
                        for b4 in range(4):
                            h = 4 * qt + b4
                            nc.tensor.matmul(
                                av[:, 33 * b4:33 * (b4 + 1)],
                                pt[:, 128 * b4:128 * (b4 + 1)],
                                vaug[g][:, 33 * h:33 * (h + 1)],
                                start=True, stop=True)
                        rec = sb.tile([128, 4], F32, tag="rec", bufs=8,
                                      name=f"rec_{img}_{qt}_{g}")
                        nc.vector.reciprocal(rec, _ap(av, 32, [[33, 4]]))
                        nc.vector.tensor_tensor(
                            out=_ap(atc[g], 128 * qt, [[32, 4], [1, 32]]),
                            in0=_ap(av, 0, [[33, 4], [1, 32]]),
                            in1=_ap(rec, 0, [[1, 4], [0, 32]]),
                            op=ALU.mult)

                actn8 = [sb.tile([128, 2048], FP8, tag="actn", bufs=2,
                                 name=f"actn_{img}_{p}") for p in range(2)]
                for fp in range(4):
                    for Q in range(2):
                        tp = ps.tile([128, 512], BF16, tag="mm", bufs=2,
                                     name=f"tp_{img}_{fp}_{Q}")
                        for gq in range(4):
                            g = 4 * Q + gq
                            nc.tensor.transpose(
                                tp[:, 128 * gq:128 * (gq + 1)],
                                atc[g][:, 128 * fp:128 * (fp + 1)], idm)
                        dst = actn8[fp // 2][:, 1024 * (fp % 2) + 512 * Q:
                                             1024 * (fp % 2) + 512 * (Q + 1)]
                        if fp % 2 == 0:
                            nc.vector.tensor_copy(dst, tp)
                        else:
                            nc.scalar.copy(dst, tp)

                for th in range(2):
                    for fo in range(4):
                        mm = ps.tile([128, 512], F32, tag="mm", bufs=2,
                                     name=f"pj_{img}_{fo}_{th}")
                        for p in range(2):
                            nc.tensor.matmul(
                                mm,
                                _ap(wproj[p], 128 * fo, [[512, 2], [1, 128]]),
                                _ap(actn8[p], 512 * th, [[1024, 2], [1, 512]]),
                                start=(p == 0), stop=(p == 1), perf_mode=DR)
                        xv = xc[fo][:, 512 * th:512 * (th + 1)]
                        nc.vector.affine_then_add(
                            xv, mm, xv, scale=ISC,
                            bias=pb_t[:, fo:fo + 1])

                with tc.high_priority():
                    r2_bc, mur2_bc = layernorm(xc, f"l2_{img}")
                z28 = z_quant(xc, r2_bc, mur2_bc, f"l2_{img}")
                return z28

            # ---------------- stage C: MLP + store ----------------
            def stage_c(img, xc, z28):
                xout = [sb.tile([128, 1024], BF16, tag="xout", bufs=4,
                                name=f"xo_{img}_{c4}") for c4 in range(4)]
                for th in range(2):
                    g8 = []
                    for j in range(8):
                        gt = sb.tile([128, 1024], FP8, tag="g8", bufs=8,
                                     name=f"g8_{img}_{th}_{j}")
                        g8.append(gt)
                    for f16 in range(16):
                        mm = ps.tile([128, 512], F32, tag="mm", bufs=2,
                                     name=f"m1_{img}_{th}_{f16}")
                        for p in range(2):
                            nc.tensor.matmul(
                                mm,
                                _ap(wm1[p], 128 * f16, [[2048, 2], [1, 128]]),
                                _ap(z28, 2048 * p + 512 * th,
                                    [[1024, 2], [1, 512]]),
                                start=(p == 0), stop=(p == 1), perf_mode=DR)
                        nc.scalar.activation(
                            g8[f16 // 2][:, 512 * (f16 % 2):
                                         512 * (f16 % 2 + 1)],
                            mm, AF.Gelu, bias=d1_t[:, f16:f16 + 1],
                            scale=ISC)
                    for fo in range(4):
                        mm2 = ps.tile([128, 512], F32, tag="st", bufs=4,
                                      name=f"m2_{img}_{th}_{fo}")
                        for j in range(8):
                            nc.tensor.matmul(
                                mm2,
                                _ap(wm2[j], 128 * fo, [[512, 2], [1, 128]]),
                                _ap(g8[j], 0, [[512, 2], [1, 512]]),
                                start=(j == 0), stop=(j == 7), perf_mode=DR)
                        xv = xc[fo][:, 512 * th:512 * (th + 1)]
                        with tc.high_priority():
                            nc.vector.affine_then_add(
                                xout[fo][:, 512 * th:512 * (th + 1)],
                                mm2, xv, scale=ISC,
                                bias=b2_t[:, fo:fo + 1])

                for hf in range(2):
                    xor = sb.tile([128, 2048], BF16, tag="xraw", bufs=3,
                                  name=f"xor_{img}_{hf}")
                    for cc in range(2):
                        nc.vector.tensor_copy(
                            _ap(xor, 1024 * cc, RAS8),
                            _ap(xout[2 * hf + cc], 0, WIN8))
                    nc.gpsimd.dma_start(
                        out=bass.AP(
                            tensor=out_d.tensor,
                            offset=out_d.offset + img * C * H * W
                            + hf * 2 * 131072,
                            ap=[[1024, 128], [131072, 2], [1, 1024]]),
                        in_=xor)

            # ---------------- skewed pipeline ----------------
            st = {}
            st[0] = stage_a(0)
            load_late_weights()
            if BI > 1:
                st[1] = stage_a(1)
            z28s = {}
            z28s[0] = stage_b(0, *st[0])
            for i in range(BI):
                if i + 2 < BI:
                    st[i + 2] = stage_a(i + 2)
                if i + 1 < BI:
                    z28s[i + 1] = stage_b(i + 1, *st[i + 1])
                stage_c(i, st[i][0], z28s[i])

    nc.compile()
    return nc


def _pair_pack(wT, nk_pairs):
    K, F = wT.shape
    assert K == nk_pairs * 256
    out = np.empty((nk_pairs, 128, 2 * F), dtype=E4)
    for p in range(nk_pairs):
        out[p, :, 0:F] = wT[256 * p:256 * p + 128, :].astype(E4)
        out[p, :, F:2 * F] = wT[256 * p + 128:256 * p + 256, :].astype(E4)
    return out


def _prep_weights(inputs):
    g1 = np.asarray(inputs["norm1_w"], np.float32)
    b1 = np.asarray(inputs["norm1_b"], np.float32)
    g2 = np.asarray(inputs["norm2_w"], np.float32)
    b2n = np.asarray(inputs["norm2_b"], np.float32)
    wqkv = np.array(inputs["qkv_w"], np.float32)
    bqkv = np.array(inputs["qkv_b"], np.float32)
    scale = HD ** -0.5
    wqkv[:C] *= scale
    bqkv = bqkv.copy()
    bqkv[:C] *= scale
    dqkv = wqkv @ b1 + bqkv
    wqkvT = (wqkv * g1[None, :]).T * WSC

    wproj = np.asarray(inputs["proj_w"], np.float32)
    dv = dqkv[2 * C:]
    pb = np.asarray(inputs["proj_b"], np.float32) + wproj @ dv
    wm1 = np.asarray(inputs["mlp_w1"], np.float32)
    d1 = wm1 @ b2n + np.asarray(inputs["mlp_b1"], np.float32)
    wm1T = (wm1 * g2[None, :]).T * WSC
    wm2 = np.asarray(inputs["mlp_w2"], np.float32)
    b2o = np.asarray(inputs["mlp_b2"], np.float32)

    rpb = np.asarray(inputs["rpb_table"], np.float32)
    rel = _relative_position_index(WS)
    bias = rpb[rel.reshape(-1)].reshape(N, N, NH)
    # additive score bias (pre-scaled by 64*64), off-window -> -inf-ish
    mask = np.full((128, NH, 128), -20.0 * WSC * WSC, np.float32)
    for wdx in range(8):
        mask[16 * wdx:16 * (wdx + 1), :, 16 * wdx:16 * (wdx + 1)] = \
            bias.transpose(1, 2, 0) * (WSC * WSC)
    # regroup into per-qt [128, 4heads*128q] tiles side by side
    m4 = mask.reshape(128, 4, 4, 128).transpose(1, 0, 2, 3)
    mask2d = np.ascontiguousarray(m4.reshape(4, 128, 512)
                                  .transpose(1, 0, 2).reshape(128, 2048))

    return {
        "wqkv": _pair_pack(np.ascontiguousarray(wqkvT), 2),
        "wproj": _pair_pack(np.ascontiguousarray(wproj.T * WSC), 2),
        "wm1": _pair_pack(np.ascontiguousarray(wm1T), 2),
        "wm2": _pair_pack(np.ascontiguousarray(wm2.T * WSC), 8),
        "dq64": np.ascontiguousarray(
            (WSC * dqkv[:C]).reshape(4, 128).T).astype(np.float32),
        "pb": np.ascontiguousarray(pb.reshape(4, 128).T).astype(np.float32),
        "d1": np.ascontiguousarray(d1.reshape(16, 128).T).astype(np.float32),
        "b2": np.ascontiguousarray(b2o.reshape(4, 128).T).astype(np.float32),
        "mask": mask2d.astype(BF),
        "idm": np.eye(128, dtype=BF),
    }


def get_program():
    if "nc" not in _cache:
        _cache["nc"] = _build_program()
    return _cache["nc"]


def make_in_maps(inputs):
    wmaps = _prep_weights(inputs)
    x_full = np.asarray(inputs["x"], np.float32)
    in_maps = []
    for core in range(NCORES):
        m = dict(wmaps)
        m["x"] = np.ascontiguousarray(x_full[BI * core:BI * (core + 1)])
        in_maps.append(m)
    return in_maps


def kernel(**inputs):
    nc = get_program()
    in_maps = make_in_maps(inputs)
    res = run_bass_kernel_spmd(nc, in_maps, list(range(NCORES)))
    out = np.concatenate([res.results[c]["out"] for c in range(NCORES)],
                         axis=0)
    return out


# revision 11
# speedup vs baseline: 1.0160x; 1.0054x over previous
"""Swin-style windowed-attention block (LN->W-MSA->residual->LN->MLP->residual)
for TRN2, data-parallel over batch across 8 NeuronCores.

Design (vs. the f32/bf16 channel-major baseline):
- 3-stage skewed software pipeline (A=load/LN1/qkv, B=attention/proj/LN2,
  C=MLP/store) emitted A0 A1 B0 | A2 B1 C0 | A3 B2 C1 | B3 C2 | C3 so the
  ACT engine's exp and gelu streams stay back-to-back across images.
- fp8e4m3 DoubleRow matmuls (2 k-tiles per instruction, 0.5 cyc/row) for
  qkv/v/proj/mlp1/mlp2; weights scaled x64 host-side into DoubleRow pair
  tiles; descales folded into activation scale slots (exp scale 1/4096
  absorbs the q64*k64 scale, gelu scale 1/64, affine_then_add scale 1/64).
- bf16 residual stream (4x DVE rate on the LN square/z passes); fp8 q/k
  tiles (halves SBUF + DMA traffic for the qh/kh head rearrange).
- relative-position bias + window mask folded into the score PSUM via one
  identity-stationary matmul per tile (off-window = -20 -> exp ~ 0), so
  softmax is exp-only on ACT with no mask multiply on DVE.
- k-bias dropped (cancels in softmax over keys); v-bias folded into the
  proj bias (softmax weights sum to 1); LN gamma/beta folded into the
  following matmul weights/biases.
- LayerNorm: ones-matmul stats on PE, Square/Sqrt+reciprocal soup (one
  act-table set; Gelu/Exp/Sqrt sets cost 1.28us per switch), r/mur rows
  broadcast across partitions via gpsimd partition_broadcast (input must
  be on partition 0 - the Q7 kernel ignores the AP base partition).
- residual updates via the fused affine_then_add custom DVE op; window
  permutation via 4-dim strided access patterns on DVE copies; casting
  DMAs (f32->bf16 loads, bf16->fp8 z/gelu quantization, bf16->f32 store)
  batched on the gpsimd SWDGE queue.
Note: TensorScalarPtr/TensorTensor ops do NOT lower for the Pool engine
(walrus ISA check) even though CoreSim accepts them - Pool runs only DMAs,
partition_broadcast and real ISA library ops.
"""
import sys
import numpy as np
import ml_dtypes

sys.path.insert(0, "/opt/trn_rl_repo")

import concourse.bass as bass
import concourse.bacc as bacc
import concourse.tile as tile
from concourse import mybir
from concourse.bass_utils import run_bass_kernel_spmd

F32 = mybir.dt.float32
BF16 = mybir.dt.bfloat16
FP8 = mybir.dt.float8e4
AF = mybir.ActivationFunctionType
ALU = mybir.AluOpType
DR = mybir.MatmulPerfMode.DoubleRow
BF = ml_dtypes.bfloat16
E4 = ml_dtypes.float8_e4m3

B, C, H, W = 32, 512, 32, 32
NH, WS = 16, 4
HD = C // NH
N = WS * WS
EPS = 1e-5
MLP_H = 4 * C
NCORES = 8
BI = B // NCORES
T = H * W

WSC = 64.0
ISC = 1.0 / WSC

_cache = {}


def _relative_position_index(ws):
    coords = np.stack(np.meshgrid(np.arange(ws), np.arange(ws), indexing="ij"))
    cf = coords.reshape(2, -1)
    rel = cf[:, :, None] - cf[:, None, :]
    rel = rel.transpose(1, 2, 0).astype(np.int64)
    rel[:, :, 0] += ws - 1
    rel[:, :, 1] += ws - 1
    rel[:, :, 0] *= 2 * ws - 1
    return rel.sum(-1)


def _ap(t, off, dims):
    return bass.AP(tensor=t.tensor, offset=t.offset + off,
                   ap=[t.ap[0]] + [list(d) for d in dims])


def _bc(t, n):
    return bass.AP(tensor=t.tensor, offset=t.offset,
                   ap=[list(t.ap[0]), [0, n]])


def _build_program():
    nc = bacc.Bacc("TRN2", target_bir_lowering=False, debug=False,
                   enable_asserts=True, num_devices=NCORES)

    def din(name, shape, dt):
        return nc.dram_tensor(name, shape, dt, kind="ExternalInput").ap()

    x_d = din("x", (BI, C, H, W), F32)
    wqkv_d = din("wqkv", (2, 128, 2 * 3 * C), FP8)
    wproj_d = din("wproj", (2, 128, 2 * C), FP8)
    wm1_d = din("wm1", (2, 128, 2 * MLP_H), FP8)
    wm2_d = din("wm2", (8, 128, 2 * C), FP8)
    dq64_d = din("dq64", (128, 4), F32)
    pb_d = din("pb", (128, 4), F32)
    d1_d = din("d1", (128, 16), F32)
    b2_d = din("b2", (128, 4), F32)
    mask_d = din("mask", (128, 4 * 512), BF16)
    idm_d = din("idm", (128, 128), BF16)

    out_d = nc.dram_tensor("out", (BI, C, H, W), F32,
                           kind="ExternalOutput").ap()

    with tile.TileContext(nc) as tc:
        with tc.tile_pool(name="sb", bufs=1) as sb, \
             tc.tile_pool(name="ps", bufs=1, space="PSUM") as ps:

            # ---------------- resident constants ----------------
            wqkv = []
            for p in range(2):
                t = sb.tile([128, 2 * 3 * C], FP8, tag="wqkv", bufs=2,
                            name=f"wqkv{p}")
                nc.sync.dma_start(out=t, in_=wqkv_d[p])
                wqkv.append(t)
            wproj = []
            wm1 = []
            wm2 = []
            mask_holder = []

            def load_late_weights():
                for p in range(2):
                    t = sb.tile([128, 2 * C], FP8, tag="wproj", bufs=2,
                                name=f"wproj{p}")
                    nc.sync.dma_start(out=t, in_=wproj_d[p])
                    wproj.append(t)
                mask_t = sb.tile([128, 4 * 512], BF16, tag="mask",
                                 name="mask_t")
                nc.sync.dma_start(out=mask_t, in_=mask_d)
                mask_holder.append(mask_t)
                for p in range(2):
                    t = sb.tile([128, 2 * MLP_H], FP8, tag="wm1", bufs=2,
                                name=f"wm1{p}")
                    nc.sync.dma_start(out=t, in_=wm1_d[p])
                    wm1.append(t)
                for p in range(8):
                    t = sb.tile([128, 2 * C], FP8, tag="wm2", bufs=8,
                                name=f"wm2{p}")
                    nc.sync.dma_start(out=t, in_=wm2_d[p])
                    wm2.append(t)
            idm = sb.tile([128, 128], BF16, tag="idm", name="idm")
            nc.sync.dma_start(out=idm, in_=idm_d)
            dq_t = sb.tile([128, 4], F32, tag="dq", name="dq_t")
            nc.sync.dma_start(out=dq_t, in_=dq64_d)
            pb_t = sb.tile([128, 4], F32, tag="pbt", name="pb_t")
            nc.sync.dma_start(out=pb_t, in_=pb_d)
            d1_t = sb.tile([128, 16], F32, tag="d1t", name="d1_t")
            nc.sync.dma_start(out=d1_t, in_=d1_d)
            b2_t = sb.tile([128, 4], F32, tag="b2t", name="b2_t")
            nc.sync.dma_start(out=b2_t, in_=b2_d)
            onesb = sb.tile([128, 1], BF16, tag="onesb", name="onesb")
            nc.vector.memset(onesb, 1.0)
            onesf = sb.tile([128, 1], F32, tag="onesf", name="onesf")
            nc.vector.memset(onesf, 1.0)
            epsb = sb.tile([1, 1], F32, tag="epsb", name="epsb")
            nc.vector.memset(epsb, EPS)

            WIN8 = [[128, 8], [16, 8], [4, 4], [1, 4]]
            RAS8 = [[128, 8], [4, 8], [32, 4], [1, 4]]

            def layernorm(xc, sfx):
                rowsb = sb.tile([1, 1024], BF16, tag="rowsb", bufs=2,
                                name=f"rowsb_{sfx}")
                rowsm = sb.tile([1, 1024], BF16, tag="rowsm", bufs=2,
                                name=f"rowsm_{sfx}")
                x2 = {}
                for hh in range(2):
                    for c4 in range(4):
                        x2t = sb.tile([128, 512], BF16, tag="x2", bufs=10,
                                      name=f"x2_{sfx}_{hh}_{c4}")
                        nc.vector.tensor_mul(
                            x2t, xc[c4][:, 512 * hh:512 * (hh + 1)],
                            xc[c4][:, 512 * hh:512 * (hh + 1)])
                        x2[(hh, c4)] = x2t
                for hh in range(2):
                    sx = ps.tile([1, 512], F32, tag="av", bufs=2,
                                 name=f"sx_{sfx}_{hh}")
                    sx2 = ps.tile([1, 512], F32, tag="av", bufs=2,
                                  name=f"sx2_{sfx}_{hh}")
                    sl0 = slice(512 * hh, 512 * (hh + 1))
                    for c4 in range(4):
                        nc.tensor.matmul(sx[0:1, :], onesb,
                                         xc[c4][:, sl0],
                                         start=(c4 == 0), stop=(c4 == 3))
                    for c4 in range(4):
                        nc.tensor.matmul(sx2[0:1, :], onesb, x2[(hh, c4)],
                                         start=(c4 == 0), stop=(c4 == 3))
                    sl = slice(512 * hh, 512 * (hh + 1))
                    with tc.high_priority():
                        mu2 = sb.tile([1, 512], F32, tag="tsq", bufs=2,
                                      name=f"mu2_{sfx}_{hh}")
                        nc.scalar.activation(mu2, sx[0:1, :], AF.Square,
                                             scale=1.0 / C)
                        varc = sb.tile([1, 512], F32, tag="varc", bufs=2,
                                       name=f"varc_{sfx}_{hh}")
                        nc.vector.scalar_tensor_tensor(
                            varc, sx2[0:1, :], 1.0 / C, mu2,
                            ALU.mult, ALU.subtract)
                        sdev = sb.tile([1, 512], F32, tag="rln", bufs=2,
                                       name=f"sdev_{sfx}_{hh}")
                        nc.scalar.activation(sdev, varc, AF.Sqrt, bias=epsb)
                        with nc.allow_low_precision(
                                reason="bf16 r/mur rows are intentional"):
                            nc.vector.reciprocal(rowsb[0:1, sl], sdev)
                            nc.vector.scalar_tensor_tensor(
                                rowsm[0:1, sl], sx[0:1, :], 1.0 / C,
                                rowsb[0:1, sl], ALU.mult, ALU.mult)
                rm_bc = sb.tile([128, 2048], BF16, tag="rmbc", bufs=2,
                                name=f"rmbc_{sfx}")
                nc.gpsimd.partition_broadcast(rm_bc[:, 0:1024],
                                              rowsb[0:1, :], channels=128)
                nc.gpsimd.partition_broadcast(rm_bc[:, 1024:2048],
                                              rowsm[0:1, :], channels=128)
                return rm_bc[:, 0:1024], rm_bc[:, 1024:2048]

            def z_quant(xc, r_bc, mur_bc, sfx):
                z8 = sb.tile([128, 4096], FP8, tag="z8p", bufs=3,
                             name=f"z8_{sfx}")
                for hf in range(2):
                    zb = sb.tile([128, 2048], BF16, tag="zb", bufs=2,
                                 name=f"zb_{sfx}_{hf}")
                    for cc in range(2):
                        c4 = 2 * hf + cc
                        t1 = sb.tile([128, 1024], BF16, tag="zt1", bufs=1,
                                     name=f"t1_{sfx}_{c4}")
                        nc.vector.tensor_mul(t1, xc[c4], r_bc)
                        nc.vector.tensor_tensor(
                            out=zb[:, 1024 * cc:1024 * (cc + 1)],
                            in0=t1, in1=mur_bc, op=ALU.subtract)
                    nc.gpsimd.dma_start(
                        out=z8[:, 2048 * hf:2048 * (hf + 1)], in_=zb)
                return z8

            # ---------------- stage A: load + LN1 + qkv ----------------
            def stage_a(img):
                xc = []
                for hf in range(2):
                    xrw = sb.tile([128, 2048], BF16, tag="xraw", bufs=3,
                                  name=f"xr_{img}_{hf}")
                    nc.gpsimd.dma_start(
                        out=xrw,
                        in_=bass.AP(
                            tensor=x_d.tensor,
                            offset=x_d.offset + img * C * H * W
                            + hf * 2 * 131072,
                            ap=[[1024, 128], [131072, 2], [1, 1024]]))
                    for cc in range(2):
                        c4 = 2 * hf + cc
                        xt = sb.tile([128, 1024], BF16, tag="xc", bufs=12,
                                     name=f"x_{img}_{c4}")
                        nc.vector.tensor_copy(_ap(xt, 0, WIN8),
                                              _ap(xrw, 1024 * cc, RAS8))
                        xc.append(xt)
                with tc.high_priority():
                    r_bc, mur_bc = layernorm(xc, f"l1_{img}")
                z8 = z_quant(xc, r_bc, mur_bc, f"l1_{img}")

                # qkv: q/k fp8 f-tiles (channel-major)
                qk = {}
                for fi in (0, 4, 1, 5, 2, 6, 3, 7):
                    qkt = sb.tile([128, 1024], FP8, tag="qk", bufs=10,
                                  name=f"qk_{img}_{fi}")
                    for th in range(2):
                        mm = ps.tile([128, 512], F32, tag="mm", bufs=2,
                                     name=f"qkp_{img}_{fi}_{th}")
                        for p in range(2):
                            nc.tensor.matmul(
                                mm,
                                _ap(wqkv[p], 128 * fi, [[1536, 2], [1, 128]]),
                                _ap(z8, 2048 * p + 512 * th,
                                    [[1024, 2], [1, 512]]),
                                start=(p == 0), stop=(p == 1), perf_mode=DR)
                        with tc.high_priority():
                            if fi < 4:
                                nc.scalar.activation(
                                    qkt[:, 512 * th:512 * (th + 1)], mm,
                                    AF.Identity, bias=dq_t[:, fi:fi + 1])
                            else:
                                nc.vector.tensor_copy(
                                    qkt[:, 512 * th:512 * (th + 1)], mm)
                    qk[fi] = qkt

                # v (token-major bf16 + interleaved ones column)
                vaug = []
                for g in range(8):
                    mm = ps.tile([128, 512], F32, tag="mm", bufs=2,
                                 name=f"vp_{img}_{g}")
                    for p in range(2):
                        nc.tensor.matmul(
                            mm,
                            _ap(z8, 2048 * p + 128 * g, [[1024, 2], [1, 128]]),
                            _ap(wqkv[p], 2 * C, [[1536, 2], [1, 512]]),
                            start=(p == 0), stop=(p == 1), perf_mode=DR)
                    va = sb.tile([128, 33 * NH], BF16, tag="vaug", bufs=16,
                                 name=f"va_{img}_{g}")
                    nc.vector.memset(_ap(va, 32, [[33, NH]]), 1.0)
                    nc.scalar.mul(
                        _ap(va, 0, [[33, NH], [1, 32]]),
                        _ap(mm, 0, [[32, NH], [1, 32]]), ISC)
                    vaug.append(va)
                return xc, qk, vaug

            # ------- stage B: attention + proj + LN2 + z28 -------
            def stage_b(img, xc, qk, vaug):
                atc = [sb.tile([128, 512], BF16, tag="atc", bufs=8,
                               name=f"atc_{img}_{g}") for g in range(8)]
                for qt in range(4):
                    qh = sb.tile([32, 4096], FP8, tag="qh", bufs=3,
                                 name=f"qh_{img}_{qt}")
                    kh = sb.tile([32, 4096], FP8, tag="kh", bufs=3,
                                 name=f"kh_{img}_{qt}")
                    for b4 in range(4):
                        nc.sync.dma_start(
                            out=qh[0:32, 1024 * b4:1024 * (b4 + 1)],
                            in_=qk[qt][32 * b4:32 * (b4 + 1), :])
                        nc.sync.dma_start(
                            out=kh[0:32, 1024 * b4:1024 * (b4 + 1)],
                            in_=qk[4 + qt][32 * b4:32 * (b4 + 1), :])
                    for g in range(8):
                        stp = ps.tile([128, 512], F32, tag="st", bufs=4,
                                      name=f"stp_{img}_{qt}_{g}")
                        nc.tensor.matmul(
                            stp, idm,
                            mask_holder[0][:, 512 * qt:512 * (qt + 1)],
                            start=True, stop=False, skip_group_check=True)
                        for b4 in range(4):
                            sl = slice(1024 * b4 + 128 * g,
                                       1024 * b4 + 128 * (g + 1))
                            nc.tensor.matmul(
                                stp[:, 128 * b4:128 * (b4 + 1)],
                                kh[0:32, sl], qh[0:32, sl],
                                start=False, stop=(b4 == 3),
                                skip_group_check=True)
                        pt = sb.tile([128, 512], BF16, tag="pt", bufs=4,
                                     name=f"pt_{img}_{qt}_{g}")
                        nc.scalar.activation(pt, stp, AF.Exp,
                                             scale=1.0 / (WSC * WSC))
                        av = ps.tile([128, 132], F32, tag="av", bufs=2,
                                     name=f"av_{img}_{qt}_{g}")
                        for b4 in range(4):
                            h = 4 * qt + b4
                            nc.tensor.matmul(
                                av[:, 33 * b4:33 * (b4 + 1)],
                                pt[:, 128 * b4:128 * (b4 + 1)],
                                vaug[g][:, 33 * h:33 * (h + 1)],
                                start=True, stop=True)
                        rec = sb.tile([128, 4], F32, tag="rec", bufs=8,
                                      name=f"rec_{img}_{qt}_{g}")
                        nc.vector.reciprocal(rec, _ap(av, 32, [[33, 4]]))
                        nc.vector.tensor_tensor(
                            out=_ap(atc[g], 128 * qt, [[32, 4], [1, 32]]),
                            in0=_ap(av, 0, [[33, 4], [1, 32]]),
                            in1=_ap(rec, 0, [[1, 4], [0, 32]]),
                            op=ALU.mult)

                actn8 = [sb.tile([128, 2048], FP8, tag="actn", bufs=2,
                                 name=f"actn_{img}_{p}") for p in range(2)]
                for fp in range(4):
                    for Q in range(2):
                        tp = ps.tile([128, 512], BF16, tag="mm", bufs=2,
                                     name=f"tp_{img}_{fp}_{Q}")
                        for gq in range(4):
                            g = 4 * Q + gq
                            nc.tensor.transpose(
                                tp[:, 128 * gq:128 * (gq + 1)],
                                atc[g][:, 128 * fp:128 * (fp + 1)], idm)
                        dst = actn8[fp // 2][:, 1024 * (fp % 2) + 512 * Q:
                                             1024 * (fp % 2) + 512 * (Q + 1)]
                        if fp % 2 == 0:
                            nc.vector.tensor_copy(dst, tp)
                        else:
                            nc.scalar.copy(dst, tp)

                for th in range(2):
                    for fo in range(4):
                        mm = ps.tile([128, 512], F32, tag="mm", bufs=2,
                                     name=f"pj_{img}_{fo}_{th}")
                        for p in range(2):
                            nc.tensor.matmul(
                                mm,
                                _ap(wproj[p], 128 * fo, [[512, 2], [1, 128]]),
                                _ap(actn8[p], 512 * th, [[1024, 2], [1, 512]]),
                                start=(p == 0), stop=(p == 1), perf_mode=DR)
                        xv = xc[fo][:, 512 * th:512 * (th + 1)]
                        nc.vector.affine_then_add(
                            xv, mm, xv, scale=ISC,
                            bias=pb_t[:, fo:fo + 1])

                with tc.high_priority():
                    r2_bc, mur2_bc = layernorm(xc, f"l2_{img}")
                z28 = z_quant(xc, r2_bc, mur2_bc, f"l2_{img}")
                return z28

            # ---------------- stage C: MLP + store ----------------
            def stage_c(img, xc, z28):
                xout = [sb.tile([128, 1024], BF16, tag="xout", bufs=4,
                                name=f"xo_{img}_{c4}") for c4 in range(4)]
                for th in range(2):
                    g8 = []
                    for j in range(8):
                        gt = sb.tile([128, 1024], FP8, tag="g8", bufs=8,
                                     name=f"g8_{img}_{th}_{j}")
                        g8.append(gt)
                    for f16 in range(16):
                        mm = ps.tile([128, 512], F32, tag="mm", bufs=2,
                                     name=f"m1_{img}_{th}_{f16}")
                        for p in range(2):
                            nc.tensor.matmul(
                                mm,
                                _ap(wm1[p], 128 * f16, [[2048, 2], [1, 128]]),
                                _ap(z28, 2048 * p + 512 * th,
                                    [[1024, 2], [1, 512]]),
                                start=(p == 0), stop=(p == 1), perf_mode=DR)
                        nc.scalar.activation(
                            g8[f16 // 2][:, 512 * (f16 % 2):
                                         512 * (f16 % 2 + 1)],
                            mm, AF.Gelu, bias=d1_t[:, f16:f16 + 1],
                            scale=ISC)
                    for fo in range(4):
                        mm2 = ps.tile([128, 512], F32, tag="st", bufs=4,
                                      name=f"m2_{img}_{th}_{fo}")
                        for j in range(8):
                            nc.tensor.matmul(
                                mm2,
                                _ap(wm2[j], 128 * fo, [[512, 2], [1, 128]]),
                                _ap(g8[j], 0, [[512, 2], [1, 512]]),
                                start=(j == 0), stop=(j == 7), perf_mode=DR)
                        xv = xc[fo][:, 512 * th:512 * (th + 1)]
                        with tc.high_priority():
                            nc.vector.affine_then_add(
                                xout[fo][:, 512 * th:512 * (th + 1)],
                                mm2, xv, scale=ISC,
                                bias=b2_t[:, fo:fo + 1])

                for hf in range(2):
                    xor = sb.tile([128, 2048], BF16, tag="xraw", bufs=3,
                                  name=f"xor_{img}_{hf}")
                    for cc in range(2):
                        nc.vector.tensor_copy(
                            _ap(xor, 1024 * cc, RAS8),
                            _ap(xout[2 * hf + cc], 0, WIN8))
                    nc.gpsimd.dma_start(
                        out=bass.AP(
                            tensor=out_d.tensor,
                            offset=out_d.offset + img * C * H * W
                            + hf * 2 * 131072,
                            ap=[[1024, 128], [131072, 2], [1, 1024]]),
                        in_=xor)

            # ---------------- skewed pipeline ----------------
            st = {}
            st[0] = stage_a(0)
            load_late_weights()
            if BI > 1:
                st[1] = stage_a(1)
            z28s = {}
            z28s[0] = stage_b(0, *st[0])
            for i in range(BI):
                if i + 2 < BI:
                    st[i + 2] = stage_a(i + 2)
                if i + 1 < BI:
                    z28s[i + 1] = stage_b(i + 1, *st[i + 1])
                stage_c(i, st[i][0], z28s[i])

    nc.compile()
    return nc


def _pair_pack(wT, nk_pairs):
    K, F = wT.shape
    assert K == nk_pairs * 256
    out = np.empty((nk_pairs, 128, 2 * F), dtype=E4)
    for p in range(nk_pairs):
        out[p, :, 0:F] = wT[256 * p:256 * p + 128, :].astype(E4)
        out[p, :, F:2 * F] = wT[256 * p + 128:256 * p + 256, :].astype(E4)
    return out


def _prep_weights(inputs):
    g1 = np.asarray(inputs["norm1_w"], np.float32)
    b1 = np.asarray(inputs["norm1_b"], np.float32)
    g2 = np.asarray(inputs["norm2_w"], np.float32)
    b2n = np.asarray(inputs["norm2_b"], np.float32)
    wqkv = np.array(inputs["qkv_w"], np.float32)
    bqkv = np.array(inputs["qkv_b"], np.float32)
    scale = HD ** -0.5
    wqkv[:C] *= scale
    bqkv = bqkv.copy()
    bqkv[:C] *= scale
    dqkv = wqkv @ b1 + bqkv
    wqkvT = (wqkv * g1[None, :]).T * WSC

    wproj = np.asarray(inputs["proj_w"], np.float32)
    dv = dqkv[2 * C:]
    pb = np.asarray(inputs["proj_b"], np.float32) + wproj @ dv
    wm1 = np.asarray(inputs["mlp_w1"], np.float32)
    d1 = wm1 @ b2n + np.asarray(inputs["mlp_b1"], np.float32)
    wm1T = (wm1 * g2[None, :]).T * WSC
    wm2 = np.asarray(inputs["mlp_w2"], np.float32)
    b2o = np.asarray(inputs["mlp_b2"], np.float32)

    rpb = np.asarray(inputs["rpb_table"], np.float32)
    rel = _relative_position_index(WS)
    bias = rpb[rel.reshape(-1)].reshape(N, N, NH)
    # additive score bias (pre-scaled by 64*64), off-window -> -inf-ish
    mask = np.full((128, NH, 128), -20.0 * WSC * WSC, np.float32)
    for wdx in range(8):
        mask[16 * wdx:16 * (wdx + 1), :, 16 * wdx:16 * (wdx + 1)] = \
            bias.transpose(1, 2, 0) * (WSC * WSC)
    # regroup into per-qt [128, 4heads*128q] tiles side by side
    m4 = mask.reshape(128, 4, 4, 128).transpose(1, 0, 2, 3)
    mask2d = np.ascontiguousarray(m4.reshape(4, 128, 512)
                                  .transpose(1, 0, 2).reshape(128, 2048))

    return {
        "wqkv": _pair_pack(np.ascontiguousarray(wqkvT), 2),
        "wproj": _pair_pack(np.ascontiguousarray(wproj.T * WSC), 2),
        "wm1": _pair_pack(np.ascontiguousarray(wm1T), 2),
        "wm2": _pair_pack(np.ascontiguousarray(wm2.T * WSC), 8),
        "dq64": np.ascontiguousarray(
            (WSC * dqkv[:C]).reshape(4, 128).T).astype(np.float32),
        "pb": np.ascontiguousarray(pb.reshape(4, 128).T).astype(np.float32),
        "d1": np.ascontiguousarray(d1.reshape(16, 128).T).astype(np.float32),
        "b2": np.ascontiguousarray(b2o.reshape(4, 128).T).astype(np.float32),
        "mask": mask2d.astype(BF),
        "idm": np.eye(128, dtype=BF),
    }


def get_program():
    if "nc" not in _cache:
        _cache["nc"] = _build_program()
    return _cache["nc"]


def make_in_maps(inputs):
    wmaps = _prep_weights(inputs)
    x_full = np.asarray(inputs["x"], np.float32)
    in_maps = []
    for core in range(NCORES):
        m = dict(wmaps)
        m["x"] = np.ascontiguousarray(x_full[BI * core:BI * (core + 1)])
        in_maps.append(m)
    return in_maps


def kernel(**inputs):
    nc = get_program()
    in_maps = make_in_maps(inputs)
    res = run_bass_kernel_spmd(nc, in_maps, list(range(NCORES)))
    out = np.concatenate([res.results[c]["out"] for c in range(NCORES)],
                         axis=0)
    return out
